# revision 20
# baseline (speedup 1.0000x reference)
"""Trainium2 Bass kernel for nn_DepthwiseSeparableFusedConv2d.

Self-contained: takes FULL inputs (x [32,256,56,56] + weights), returns FULL
output [32,256,56,56].  Data-parallel over batch across 8 NeuronCores; the
QuantMeasure / RangeBN global statistics are synchronized with small
AllGather collectives.

v2 design (per core: 4 batches, channels on partitions in 2 groups of 128):
  A:  load x contiguously, per-(channel,sample) min/max
  AG1 (per-sample min/max) -> x quant params;  diag weights scaled by s_x
  B:  quantize x -> integer k (fp32 values) written into padded tiles,
      borders = -mn/s
  C:  depthwise 3x3 conv as 9 accumulating diagonal-weight float32r matmuls;
      ACT evicts PSUM (+bias fold + channel-sum accum); DVE min/max of h1
  AG2 (per-(c,sample) h1 min/max + sums) -> qm(h1) params, RangeBN1 chunk
      stats, BN1 mean (mean over quantized h1 ~= mean h1), analytic qm(h2)
      bounds -- no extra pass, no second collective
  D:  quantize h1 -> u8 k1
  E:  k2 = round(clip(affine(k1))) as fp32 integers (BN1+requant fused)
  F:  pointwise conv k2 @ (qpw*s2) in float32r; ACT evict (+sums);
      DVE min/max of h3; h3 stays in SBUF (no DRAM spill)
  AG5 -> qm(h3) params + RangeBN2 stats + BN2 mean
  G:  requantize h3 -> k3 (in place);  H: out = relu(BN2(k3)) -> DMA out
"""

import math
import numpy as np

# ---------------------------------------------------------------- constants
P = 128
G = 2                 # channel groups (256 = 2*128)
B_FULL = 32
BL = 4                # batches per core
NCORES = 8
HH = 56
IMG = HH * HH         # 3136
PADW = 58
PADI = PADW * PADW    # 3364
MAGIC = 12582912.0    # 1.5 * 2**23  (fp32 round-to-nearest-even trick)
QMAX = 255.0
N_TOT = B_FULL * IMG  # 100352
NCHUNKS = 16
EPS = 1e-5
_N_CHUNK_EL = B_FULL * IMG // NCHUNKS
SCALE_FIX = float((0.5 * 0.35) * (1 + (math.pi * math.log(4)) ** 0.5)
                  / ((2 * math.log(_N_CHUNK_EL)) ** 0.5))

_PROGRAM_CACHE = {}


def _host_quant(w):
    w = np.asarray(w, np.float32)
    mn = w.min()
    mx = w.max()
    scale = np.maximum(((mx - mn) / np.float32(QMAX)).astype(np.float32),
                       np.float32(1e-8))
    t = np.clip((w - mn) / scale, np.float32(0.0), np.float32(QMAX)).astype(np.float32)
    return (np.round(t) * scale + mn).astype(np.float32)


def build_program(limit=7):  # limit unused in full build
    import concourse.bacc as bacc
    import concourse.mybir as mybir
    import concourse.tile as tile

    f32 = mybir.dt.float32
    f32r = mybir.dt.float32r
    bf16 = mybir.dt.bfloat16
    u8 = mybir.dt.uint8
    AL = mybir.AluOpType
    AF = mybir.ActivationFunctionType
    AX = mybir.AxisListType

    nc = bacc.Bacc('TRN2', target_bir_lowering=False, debug=False,
                   num_devices=NCORES)

    # ------------------------------------------------ external tensors
    x_in = nc.dram_tensor('x', [BL, 256, HH, HH], f32, kind='ExternalInput')
    ident_in = nc.dram_tensor('ident', [P, P], f32, kind='ExternalInput')
    qdw_in = nc.dram_tensor('qdw', [G, P, 9], f32, kind='ExternalInput')
    wsum_in = nc.dram_tensor('wsum', [G, P], f32, kind='ExternalInput')
    qdb_in = nc.dram_tensor('qdb', [G, P], f32, kind='ExternalInput')
    qbn1w_in = nc.dram_tensor('qbn1w', [G, P], f32, kind='ExternalInput')
    bn1b_in = nc.dram_tensor('bn1b', [G, P], f32, kind='ExternalInput')
    qbn2w_in = nc.dram_tensor('qbn2w', [G, P], f32, kind='ExternalInput')
    bn2b_in = nc.dram_tensor('bn2b', [G, P], f32, kind='ExternalInput')
    pwsum_in = nc.dram_tensor('pwsum', [G, P], f32, kind='ExternalInput')
    # pwT[kg, cin(128), (coutg, cout)] : lhsT layout, already transposed
    pwT_in = nc.dram_tensor('pwT', [G, P, 256], f32, kind='ExternalInput')
    out_d = nc.dram_tensor('out', [BL, 256, HH, HH], f32, kind='ExternalOutput')

    rg = [list(range(NCORES))]

    # pointwise free-dim chunks: 7 x 448 (all >= 256 so f32r runs 1 cyc/col)
    PWC = 448
    PW_CHUNKS = [(i * PWC, PWC) for i in range(7)]

    with tile.TileContext(nc) as tc:
        with (
            tc.tile_pool(name='perm', bufs=1) as perm,
            tc.tile_pool(name='kpool', bufs=6) as kpool,
            tc.tile_pool(name='k2p', bufs=3) as k2p,
            tc.tile_pool(name='dram', bufs=1, space='DRAM') as dpool,
            tc.tile_pool(name='tp', bufs=2, space='PSUM') as tpp,
        ):
            # ------------------------------------------------ constants
            ident = perm.tile([P, P], f32, name='identsb')
            nc.sync.dma_start(ident[:], ident_in[:])
            # warmup collective: absorbs the CC cold-start latency
            # while phase A runs
            wu = perm.tile([1, 2], f32, name='wu')
            nc.vector.memset(wu[:], 0.0)
            ag0_in = dpool.tile([2], f32, name='ag0_in')
            ag0_out = dpool.tile([NCORES * 2], f32, name='ag0_out')
            nc.sync.dma_start(ag0_in[None, :], wu[:])
            nc.gpsimd.collective_compute(
                'AllGather', AL.bypass, replica_groups=rg,
                ins=[ag0_in[:].opt()], outs=[ag0_out[:].opt()])
            qdw = perm.tile([P, G, 9], f32, name='qdwsb')
            nc.sync.dma_start(qdw[:], qdw_in.rearrange('g c t -> c g t'))

            def load_gp(t_in, nm):
                t = perm.tile([P, G], f32, name=nm)
                nc.sync.dma_start(t[:], t_in.rearrange('g c -> c g'))
                return t
            wsum_t = load_gp(wsum_in, 'wsumsb')
            qdb_t = load_gp(qdb_in, 'qdbsb')
            qbn1w_t = load_gp(qbn1w_in, 'qbn1wsb')
            bn1b_t = load_gp(bn1b_in, 'bn1bsb')
            qbn2w_t = load_gp(qbn2w_in, 'qbn2wsb')
            bn2b_t = load_gp(bn2b_in, 'bn2bsb')
            pwsum_t = load_gp(pwsum_in, 'pwsumsb')
            pwT = perm.tile([P, G, 256], f32, name='pwTsb')
            nc.sync.dma_start(pwT[:], pwT_in[:].rearrange('g c m -> c g m'))

            # diag weight matrices (filled after AG1: qdw * s_x folded in)
            diagt = perm.tile([P, G * 9 * P], f32r, name='diagt')

            # ------------------------------------------------ stat tiles
            # per-(c,b): [min(BL) | max(BL) | sum(BL)]
            xstat = [perm.tile([P, 2 * BL], f32, name=f'xstat{g}')
                     for g in range(G)]
            h1stat = [perm.tile([P, 3 * BL], f32, name=f'h1stat{g}')
                      for g in range(G)]
            h3stat = [perm.tile([P, 3 * BL], f32, name=f'h3stat{g}')
                      for g in range(G)]
            hsum8 = [perm.tile([P, BL, 8], f32, name=f'hsum8_{g}')
                     for g in range(G)]
            Ag = [perm.tile([P, 3, NCORES, 2, 2], f32, name=f'Ag{g}')
                  for g in range(G)]
            cstat = [perm.tile([P, 2, NCORES * 2], f32, name=f'cstat{g}')
                     for g in range(G)]

            def sc1(nm):
                return perm.tile([1, 1], f32, name=nm)

            def bc1(nm):
                return perm.tile([P, 1], f32, name=nm)

            # scalar math helper: from sum-of-mins/sum-of-maxes -> quant params
            def quant_params(mnsum, mxsum, tag):
                mn = sc1(f'mn_{tag}')
                mx = sc1(f'mx_{tag}')
                nc.vector.tensor_scalar(mn[:], mnsum[:], 1.0 / B_FULL, None, op0=AL.mult)
                nc.vector.tensor_scalar(mx[:], mxsum[:], 1.0 / B_FULL, None, op0=AL.mult)
                d = sc1(f'd_{tag}')
                nc.vector.tensor_sub(d[:], mx[:], mn[:])
                s = sc1(f's_{tag}')
                nc.vector.tensor_scalar(s[:], d[:], 1.0 / QMAX, 1e-8,
                                        op0=AL.mult, op1=AL.max)
                inv_s = sc1(f'invs_{tag}')
                nc.vector.reciprocal(inv_s[:], s[:])
                negmn = sc1(f'negmn_{tag}')
                nc.vector.tensor_scalar(negmn[:], mn[:], -1.0, None, op0=AL.mult)
                bias = sc1(f'bias_{tag}')
                nc.vector.tensor_mul(bias[:], negmn[:], inv_s[:])
                return {'mn': mn, 'mx': mx, 's': s, 'inv_s': inv_s,
                        'negmn': negmn, 'bias': bias}

            def bcast(src, nm):
                t = bc1(nm)
                nc.gpsimd.partition_broadcast(t[:], src[:])
                return t

            # quantize chain on a small [P, F] stat tile (value form k, fp32)
            def qchain_small(ap, inv_s_bc, bias_bc):
                nc.scalar.activation(ap, ap, AF.Relu, bias=bias_bc[:, 0:1],
                                     scale=inv_s_bc[:, 0:1])
                nc.vector.tensor_scalar(ap, ap, QMAX, MAGIC, op0=AL.min, op1=AL.add)
                nc.vector.tensor_scalar(ap, ap, MAGIC, None, op0=AL.subtract)

            # fast min+max of a [P, IMG] tile: tensor_tensor folding tree
            # (DVE elementwise runs ~3x faster than tensor_reduce per el)
            def minmax_tree(nm, src_ap, mn_ap, mx_ap):
                scr = img.tile([P, IMG // 2], f32, name=nm, tag='img')
                L1, L2, L3 = IMG // 2, IMG // 4, IMG // 8
                for op, o_ap in ((AL.min, mn_ap), (AL.max, mx_ap)):
                    nc.vector.tensor_tensor(scr[:, 0:L1], src_ap[:, 0:L1],
                                            src_ap[:, L1:IMG], op=op)
                    nc.vector.tensor_tensor(scr[:, 0:L2], scr[:, 0:L2],
                                            scr[:, L2:L1], op=op)
                    nc.vector.tensor_tensor(scr[:, 0:L3], scr[:, 0:L3],
                                            scr[:, L3:L2], op=op)
                    nc.vector.tensor_reduce(o_ap, scr[:, 0:L3],
                                            axis=AX.X, op=op)

            # =================================================================
            # Stage A: load x (contiguous) + x stats
            # =================================================================
            with tc.tile_pool(name='img', bufs=10) as img:
                raw = {}
                for b in range(BL):
                    for g in range(G):
                        t = img.tile([P, IMG], f32, name=f'raw{g}_{b}',
                                     tag='img')
                        raw[(g, b)] = t
                        nc.sync.dma_start(
                            t[:].rearrange('p (h w) -> p h w', h=HH),
                            x_in[b, g * P:(g + 1) * P])
                        minmax_tree(f'xs{g}_{b}', t[:],
                                    xstat[g][:, b:b + 1],
                                    xstat[g][:, BL + b:BL + b + 1])

                # --- AG1: per-sample min/max (8 floats per core) ---
                tmin = tpp.tile([BL, G * P], f32, name='tmin1', tag='tp')
                tmax = tpp.tile([BL, G * P], f32, name='tmax1', tag='tp')
                for g in range(G):
                    nc.tensor.transpose(tmin[:, g * P:(g + 1) * P],
                                        xstat[g][:, 0:BL], ident[:])
                    nc.tensor.transpose(tmax[:, g * P:(g + 1) * P],
                                        xstat[g][:, BL:2 * BL], ident[:])
                ab1 = perm.tile([BL, 2], f32, name='ab1')
                nc.vector.tensor_reduce(ab1[:, 0:1], tmin[:], axis=AX.X, op=AL.min)
                nc.vector.tensor_reduce(ab1[:, 1:2], tmax[:], axis=AX.X, op=AL.max)

                ag1_in = dpool.tile([BL * 2], f32, name='ag1_in')
                ag1_out = dpool.tile([NCORES * BL * 2], f32, name='ag1_out')
                nc.sync.dma_start(ag1_in.rearrange('(b s) -> b s', s=2), ab1[:])
                nc.gpsimd.collective_compute(
                    'AllGather', AL.bypass, replica_groups=rg,
                    ins=[ag1_in[:].opt()], outs=[ag1_out[:].opt()])
                agb1 = perm.tile([1, NCORES * BL * 2], f32, name='agb1')
                nc.sync.dma_start(agb1[:], ag1_out[None, :])
                v1 = agb1.rearrange('p (cb s) -> p s cb', s=2)
                mnsum_x = sc1('mnsum_x')
                mxsum_x = sc1('mxsum_x')
                nc.vector.tensor_reduce(mnsum_x[:], v1[:, 0, :], axis=AX.X, op=AL.add)
                nc.vector.tensor_reduce(mxsum_x[:], v1[:, 1, :], axis=AX.X, op=AL.add)
                qx = quant_params(mnsum_x, mxsum_x, 'x')
                invsx_bc = bcast(qx['inv_s'], 'invsx_bc')
                biasx_bc = bcast(qx['bias'], 'biasx_bc')
                sx_bc = bcast(qx['s'], 'sx_bc')
                # centered k' = k - 128 halves |k| so f32r operand rounding
                # of the products shrinks ~5x.  border k' = -mn/s - 128;
                # wsum correction uses mn' = mn + 128*s.
                nmos128 = sc1('nmos128')
                nc.vector.tensor_scalar(nmos128[:], qx['bias'][:], -128.0,
                                        None, op0=AL.add)
                nmos_bc = bcast(nmos128, 'nmos_bc')
                mnp = sc1('mnp')
                nc.vector.tensor_scalar(mnp[:], qx['s'][:], 128.0,
                                        qx['mn'][:, 0:1],
                                        op0=AL.mult, op1=AL.add)
                mnx_bc = bcast(mnp, 'mnx_bc')

                # diag weights: ident * (qdw[c,g,t] * s_x)
                for g in range(G):
                    for t in range(9):
                        i = g * 9 + t
                        nc.vector.tensor_scalar(
                            diagt[:, i * P:(i + 1) * P], ident[:],
                            qdw[:, g, t:t + 1], sx_bc[:, 0:1],
                            op0=AL.mult, op1=AL.mult)

                # const1 = qdb + mn_x * wsum  (per channel)
                const1 = perm.tile([P, G], f32, name='const1')
                for g in range(G):
                    nc.vector.scalar_tensor_tensor(
                        const1[:, g:g + 1], wsum_t[:, g:g + 1], mnx_bc[:, 0:1],
                        qdb_t[:, g:g + 1], op0=AL.mult, op1=AL.add)

                # constant border strip: value -mn/s everywhere
                bord = perm.tile([P, PADW], f32r, name='bord')
                nc.vector.tensor_scalar(bord[:], ident[:, 0:PADW], 0.0,
                                        nmos_bc[:, 0:1],
                                        op0=AL.mult, op1=AL.add)

                # =========================================================
                # Stage B+C per tile: quantize into padded tile, conv, evict
                # =========================================================
                h1 = {}
                with tc.tile_pool(name='cv', bufs=6, space='PSUM') as cvp:
                    for b in range(BL):
                        for g in range(G):
                            rt = raw[(g, b)]
                            # B: k = round(clip((x-mn)/s)) via saturating
                            # u8 convert (RNE), then center to k-128 (f32r)
                            ku = kpool.tile([P, IMG], u8, name=f'kx{g}_{b}',
                                            tag='k8')
                            nc.scalar.activation(ku[:], rt[:], AF.Relu,
                                                 bias=biasx_bc[:, 0:1],
                                                 scale=invsx_bc[:, 0:1])
                            xp = img.tile([P, PADI], f32r, name=f'xp{g}_{b}',
                                          tag='img')
                            v = xp.rearrange('p (h w) -> p h w', h=PADW)
                            nc.vector.tensor_scalar(
                                v[:, 1:57, 1:57],
                                ku[:].rearrange('p (h w) -> p h w', h=HH),
                                128.0, None, op0=AL.subtract)
                            # borders := -mn/s (cancels wsum correction)
                            for bap, bw in ((v[:, 0, :], PADW),
                                            (v[:, 57, :], PADW),
                                            (v[:, 1:57, 0], HH),
                                            (v[:, 1:57, 57], HH)):
                                nc.vector.tensor_scalar(bap, bord[:, 0:bw],
                                                        1.0, None, op0=AL.mult)
                            # C: depthwise conv via diag f32r matmuls
                            src = v
                            h1t = img.tile([P, IMG], f32, name=f'h1_{g}_{b}',
                                           tag='img')
                            h1[(g, b)] = h1t
                            for half in range(2):
                                pst = [cvp.tile([P, 392], f32,
                                                name=f'cv{g}{b}{half}{rb}',
                                                tag='cv')
                                       for rb in range(4)]
                                for t in range(9):
                                    di, dj = t // 3, t % 3
                                    lhs = diagt[:, (g * 9 + t) * P:(g * 9 + t + 1) * P]
                                    for rb in range(4):
                                        r0 = half * 28 + rb * 7
                                        rhs = src[:, r0 + di:r0 + di + 7,
                                                  dj:dj + 56]
                                        nc.tensor.matmul(
                                            pst[rb][:], lhs, rhs,
                                            start=(t == 0), stop=(t == 8))
                                for rb in range(4):
                                    r0 = half * 28 + rb * 7
                                    j = half * 4 + rb
                                    nc.scalar.activation(
                                        h1t[:, r0 * HH:(r0 + 7) * HH],
                                        pst[rb][:], AF.Identity,
                                        bias=const1[:, g:g + 1], scale=1.0,
                                        accum_out=hsum8[g][:, b, j:j + 1])
                            minmax_tree(f'h1s{g}_{b}', h1t[:],
                                        h1stat[g][:, b:b + 1],
                                        h1stat[g][:, BL + b:BL + b + 1])
                            nc.vector.tensor_reduce(
                                h1stat[g][:, 2 * BL + b:2 * BL + b + 1],
                                hsum8[g][:, b], axis=AX.X, op=AL.add)

                # =========================================================
                # AG2: per-(channel,sample) h1 min/max/sum
                # =========================================================
                ag2_in = dpool.tile([G * P * 3 * BL], f32, name='ag2_in')
                ag2_out = dpool.tile([NCORES * G * P * 3 * BL], f32, name='ag2_out')
                v2i = ag2_in.rearrange('(g c f) -> g c f', g=G, c=P)
                for g in range(G):
                    nc.sync.dma_start(v2i[g], h1stat[g][:])
                nc.gpsimd.collective_compute(
                    'AllGather', AL.bypass, replica_groups=rg,
                    ins=[ag2_in[:].opt()], outs=[ag2_out[:].opt()])
                v2o = ag2_out.rearrange(
                    '(core g c s q b) -> g c s core q b',
                    core=NCORES, g=G, c=P, s=3, q=2)
                for g in range(G):
                    for s in range(3):
                        nc.sync.dma_start(Ag[g][:, s], v2o[g][:, s])

                # chunk stats (pair of batches within a core)
                for g in range(G):
                    nc.vector.tensor_reduce(
                        cstat[g][:, 0, :], Ag[g][:, 0], axis=AX.X, op=AL.min)
                    nc.vector.tensor_reduce(
                        cstat[g][:, 1, :], Ag[g][:, 1], axis=AX.X, op=AL.max)

                # per-sample min/max over all 256 channels -> qm() params
                def sample_params(stattiles, tag):
                    tmn = tpp.tile([B_FULL, G * P], f32, name=f'tmn_{tag}', tag='tp')
                    tmx = tpp.tile([B_FULL, G * P], f32, name=f'tmx_{tag}', tag='tp')
                    for g in range(G):
                        flat = stattiles[g].rearrange('p s core q b -> p (s core q b)')
                        nc.tensor.transpose(tmn[:, g * P:(g + 1) * P],
                                            flat[:, 0:B_FULL], ident[:])
                        nc.tensor.transpose(tmx[:, g * P:(g + 1) * P],
                                            flat[:, B_FULL:2 * B_FULL], ident[:])
                    pm = perm.tile([B_FULL, 2], f32, name=f'pm_{tag}')
                    nc.vector.tensor_reduce(pm[:, 0:1], tmn[:], axis=AX.X, op=AL.min)
                    nc.vector.tensor_reduce(pm[:, 1:2], tmx[:], axis=AX.X, op=AL.max)
                    ta = tpp.tile([1, B_FULL], f32, name=f'ta_{tag}', tag='tp')
                    tb = tpp.tile([1, B_FULL], f32, name=f'tb_{tag}', tag='tp')
                    nc.tensor.transpose(ta[:], pm[:, 0:1], ident[0:B_FULL, 0:B_FULL])
                    nc.tensor.transpose(tb[:], pm[:, 1:2], ident[0:B_FULL, 0:B_FULL])
                    mnsum = sc1(f'mnsum_{tag}')
                    mxsum = sc1(f'mxsum_{tag}')
                    nc.vector.tensor_reduce(mnsum[:], ta[:], axis=AX.X, op=AL.add)
                    nc.vector.tensor_reduce(mxsum[:], tb[:], axis=AX.X, op=AL.add)
                    return quant_params(mnsum, mxsum, tag)

                q1 = sample_params(Ag, 'h1')
                invs1_bc = bcast(q1['inv_s'], 'invs1_bc')
                bias1_bc = bcast(q1['bias'], 'bias1_bc')
                s1_bc = bcast(q1['s'], 's1_bc')
                mn1_bc = bcast(q1['mn'], 'mn1_bc')

                # RangeBN scale from chunk stats
                def rangebn_scale(cstat_g, invs_bc, bias_bc, s_bc, mn_bc, tag):
                    scpk = perm.tile([P, G], f32, name=f'scpk_{tag}')
                    for g in range(G):
                        c = cstat_g[g].rearrange('p s f -> p (s f)')
                        qchain_small(c[:, :], invs_bc, bias_bc)
                        # now c holds integer k; mean over 16 chunks, value form
                        mm = perm.tile([P, 2], f32, name=f'mm_{tag}{g}')
                        nc.vector.tensor_reduce(
                            mm[:], cstat_g[g][:], axis=AX.X, op=AL.add)
                        # mm = (sum k)/16 * s + mn
                        nc.vector.tensor_scalar(mm[:], mm[:], 1.0 / NCHUNKS,
                                                s_bc[:, 0:1],
                                                op0=AL.mult, op1=AL.mult)
                        nc.vector.tensor_scalar(mm[:], mm[:], mn_bc[:, 0:1],
                                                None, op0=AL.add)
                        d = perm.tile([P, 1], f32, name=f'dmm_{tag}{g}')
                        nc.vector.tensor_sub(d[:], mm[:, 1:2], mm[:, 0:1])
                        nc.vector.tensor_scalar(d[:], d[:], SCALE_FIX, EPS,
                                                op0=AL.mult, op1=AL.add)
                        nc.vector.reciprocal(scpk[:, g:g + 1], d[:])
                    # quantize scale over all 256 channels
                    tq = tpp.tile([1, G * P], f32, name=f'tq_{tag}', tag='tp')
                    for g in range(G):
                        nc.tensor.transpose(tq[:, g * P:(g + 1) * P],
                                            scpk[:, g:g + 1], ident[:])
                    smn = sc1(f'smn_{tag}')
                    smx = sc1(f'smx_{tag}')
                    nc.vector.tensor_reduce(smn[:], tq[:], axis=AX.X, op=AL.min)
                    nc.vector.tensor_reduce(smx[:], tq[:], axis=AX.X, op=AL.max)
                    dd = sc1(f'sd_{tag}')
                    nc.vector.tensor_sub(dd[:], smx[:], smn[:])
                    ss = sc1(f'ss_{tag}')
                    nc.vector.tensor_scalar(ss[:], dd[:], 1.0 / QMAX, 1e-8,
                                            op0=AL.mult, op1=AL.max)
                    invss = sc1(f'invss_{tag}')
                    nc.vector.reciprocal(invss[:], ss[:])
                    negsmn = sc1(f'negsmn_{tag}')
                    nc.vector.tensor_scalar(negsmn[:], smn[:], -1.0, None, op0=AL.mult)
                    bss = sc1(f'bss_{tag}')
                    nc.vector.tensor_mul(bss[:], negsmn[:], invss[:])
                    invss_bc = bcast(invss, f'invss_bc_{tag}')
                    bss_bc = bcast(bss, f'bss_bc_{tag}')
                    ss_bc = bcast(ss, f'ss_bc_{tag}')
                    smn_bc = bcast(smn, f'smn_bc_{tag}')
                    qchain_small(scpk[:, :], invss_bc, bss_bc)
                    nc.vector.tensor_scalar(scpk[:], scpk[:], ss_bc[:, 0:1],
                                            None, op0=AL.mult)
                    nc.vector.tensor_scalar(scpk[:], scpk[:], smn_bc[:, 0:1],
                                            None, op0=AL.add)
                    return scpk

                qscale1 = rangebn_scale(cstat, invs1_bc, bias1_bc, s1_bc,
                                        mn1_bc, 'bn1')
                A1 = perm.tile([P, G], f32, name='A1')
                nc.vector.tensor_mul(A1[:], qscale1[:], qbn1w_t[:])
                cA1 = perm.tile([P, G], f32, name='cA1')
                nc.vector.tensor_scalar(cA1[:], A1[:], s1_bc[:, 0:1], None,
                                        op0=AL.mult)

                # mean1 = (sum over cores+batches of h1 sums) / N_TOT
                mean1 = perm.tile([P, G], f32, name='mean1')
                for g in range(G):
                    fsum = Ag[g].rearrange('p s core q b -> p s (core q b)')
                    nc.vector.tensor_reduce(mean1[:, g:g + 1],
                                            fsum[:, 2, :], axis=AX.X, op=AL.add)
                nc.vector.tensor_scalar(mean1[:], mean1[:], 1.0 / N_TOT, None,
                                        op0=AL.mult)
                cB1 = perm.tile([P, G], f32, name='cB1')
                nc.vector.tensor_scalar(cB1[:], mean1[:], -1.0,
                                        mn1_bc[:, 0:1], op0=AL.mult, op1=AL.add)
                # cB1 currently = (mn1 - mean1); multiply by A1, add bn1b
                nc.vector.tensor_mul(cB1[:], cB1[:], A1[:])
                nc.vector.tensor_add(cB1[:], cB1[:], bn1b_t[:])

                # analytic qm(h2) bounds from Ag (monotone: cA1 >= 0)
                for g in range(G):
                    flat = Ag[g].rearrange('p s core q b -> p (s core q b)')
                    ext = flat[:, 0:2 * B_FULL]
                    qchain_small(ext, invs1_bc, bias1_bc)
                    nc.scalar.activation(ext, ext,
                                         AF.Relu, bias=cB1[:, g:g + 1],
                                         scale=cA1[:, g:g + 1])
                q2 = sample_params(Ag, 'h2')
                invs2_bc = bcast(q2['inv_s'], 'invs2_bc')
                bias2_bc = bcast(q2['bias'], 'bias2_bc')
                s2_bc = bcast(q2['s'], 's2_bc')
                mn2_bc = bcast(q2['mn'], 'mn2_bc')

                # E-stage fused affine: k2 = round(clip(aE*k1 + bE, 0, 255))
                aE = perm.tile([P, G], f32, name='aE')
                bE = perm.tile([P, G], f32, name='bE')
                nc.vector.tensor_scalar(aE[:], cA1[:], invs2_bc[:, 0:1], None,
                                        op0=AL.mult)
                nc.vector.tensor_scalar(bE[:], cB1[:], mn2_bc[:, 0:1],
                                        invs2_bc[:, 0:1],
                                        op0=AL.subtract, op1=AL.mult)

                # scaled pointwise weights (bf16: k2 integers exact, weight
                # rounding in the 256-way contraction is harmless) + const3
                pwTs = perm.tile([P, G, 256], bf16, name='pwTs')
                nc.vector.tensor_scalar(pwTs[:], pwT[:], s2_bc[:, 0:1], None,
                                        op0=AL.mult)
                const3 = perm.tile([P, G], f32, name='const3')
                nc.vector.tensor_scalar(const3[:], pwsum_t[:], mn2_bc[:, 0:1],
                                        None, op0=AL.mult)

                # =========================================================
                # Stages D+E+F per batch: h1 -> k1(u8) -> k2(u8) -> bf16
                # -> h3 (SBUF); Sum(h3) accumulated during eviction
                # =========================================================
                h3 = {}
                with tc.tile_pool(name='pw', bufs=6, space='PSUM') as pwp:
                    for b in range(BL):
                        k2b = {}
                        for g in range(G):
                            # D (DVE): k1 = round(clip((h1-mn1)/s1)) --
                            # the u8 convert does RNE + [0,255] saturation
                            ht = h1[(g, b)]
                            nc.vector.tensor_scalar(ht[:], ht[:],
                                                    invs1_bc[:, 0:1],
                                                    bias1_bc[:, 0:1],
                                                    op0=AL.mult, op1=AL.add)
                            kt = kpool.tile([P, IMG], u8, name=f'k1_{g}_{b}',
                                            tag='k8')
                            nc.vector.tensor_scalar(kt[:], ht[:], 1.0, None,
                                                    op0=AL.mult)
                            # E: k2 = round(clip(aE*k1 + bE)), in place u8->u8
                            nc.scalar.activation(kt[:], kt[:], AF.Relu,
                                                 bias=bE[:, g:g + 1],
                                                 scale=aE[:, g:g + 1])
                            # convert to bf16 for the pointwise matmuls
                            k2t = k2p.tile([P, IMG], bf16, name=f'k2_{g}_{b}',
                                           tag='k2')
                            nc.vector.tensor_scalar(k2t[:], kt[:], 1.0, None,
                                                    op0=AL.mult)
                            k2b[g] = k2t
                        # F: pointwise conv for this batch (bf16)
                        for cg in range(G):
                            h3t = img.tile([P, IMG], f32, name=f'h3_{cg}_{b}',
                                           tag='img')
                            h3[(cg, b)] = h3t
                            for blk in (PW_CHUNKS[0:4], PW_CHUNKS[4:7]):
                                pst = {}
                                for (c0, nn) in blk:
                                    pst[c0] = pwp.tile([P, PWC], f32,
                                                       name=f'pw{cg}{b}{c0}',
                                                       tag='pw')
                                for kg in range(G):
                                    lhs = pwTs[:, kg, cg * P:(cg + 1) * P]
                                    for (c0, nn) in blk:
                                        nc.tensor.matmul(
                                            pst[c0][:, 0:nn], lhs,
                                            k2b[kg][:, c0:c0 + nn],
                                            start=(kg == 0), stop=(kg == 1))
                                for ji, (c0, nn) in enumerate(blk):
                                    j = (0 if c0 < 4 * PWC else 4) + ji
                                    nc.scalar.activation(
                                        h3t[:, c0:c0 + nn], pst[c0][:, 0:nn],
                                        AF.Identity, bias=const3[:, cg:cg + 1],
                                        scale=1.0,
                                        accum_out=hsum8[cg][:, b, j:j + 1])
                            minmax_tree(f'h3s{cg}_{b}', h3t[:],
                                        h3stat[cg][:, b:b + 1],
                                        h3stat[cg][:, BL + b:BL + b + 1])
                            nc.vector.tensor_reduce(
                                h3stat[cg][:, 2 * BL + b:2 * BL + b + 1],
                                hsum8[cg][:, b, 0:7], axis=AX.X, op=AL.add)

                # =========================================================
                # AG5 + RangeBN2 stats
                # =========================================================
                ag5_in = dpool.tile([G * P * 3 * BL], f32, name='ag5_in')
                ag5_out = dpool.tile([NCORES * G * P * 3 * BL], f32,
                                     name='ag5_out')
                v5i = ag5_in.rearrange('(g c f) -> g c f', g=G, c=P)
                for g in range(G):
                    nc.sync.dma_start(v5i[g], h3stat[g][:])
                nc.gpsimd.collective_compute(
                    'AllGather', AL.bypass, replica_groups=rg,
                    ins=[ag5_in[:].opt()], outs=[ag5_out[:].opt()])
                v5o = ag5_out.rearrange(
                    '(core g c s q b) -> g c s core q b',
                    core=NCORES, g=G, c=P, s=3, q=2)
                for g in range(G):
                    for s in range(3):
                        nc.sync.dma_start(Ag[g][:, s], v5o[g][:, s])
                for g in range(G):
                    nc.vector.tensor_reduce(
                        cstat[g][:, 0, :], Ag[g][:, 0], axis=AX.X, op=AL.min)
                    nc.vector.tensor_reduce(
                        cstat[g][:, 1, :], Ag[g][:, 1], axis=AX.X, op=AL.max)
                q3 = sample_params(Ag, 'h3')
                invs3_bc = bcast(q3['inv_s'], 'invs3_bc')
                bias3_bc = bcast(q3['bias'], 'bias3_bc')
                s3_bc = bcast(q3['s'], 's3_bc')
                mn3_bc = bcast(q3['mn'], 'mn3_bc')
                qscale3 = rangebn_scale(cstat, invs3_bc, bias3_bc, s3_bc,
                                        mn3_bc, 'bn2')
                A3 = perm.tile([P, G], f32, name='A3')
                nc.vector.tensor_mul(A3[:], qscale3[:], qbn2w_t[:])
                cA3 = perm.tile([P, G], f32, name='cA3')
                nc.vector.tensor_scalar(cA3[:], A3[:], s3_bc[:, 0:1], None,
                                        op0=AL.mult)
                mean3 = perm.tile([P, G], f32, name='mean3')
                for g in range(G):
                    fsum = Ag[g].rearrange('p s core q b -> p s (core q b)')
                    nc.vector.tensor_reduce(mean3[:, g:g + 1],
                                            fsum[:, 2, :], axis=AX.X, op=AL.add)
                nc.vector.tensor_scalar(mean3[:], mean3[:], 1.0 / N_TOT, None,
                                        op0=AL.mult)
                cB3 = perm.tile([P, G], f32, name='cB3')
                nc.vector.tensor_scalar(cB3[:], mean3[:], -1.0,
                                        mn3_bc[:, 0:1], op0=AL.mult, op1=AL.add)
                nc.vector.tensor_mul(cB3[:], cB3[:], A3[:])
                nc.vector.tensor_add(cB3[:], cB3[:], bn2b_t[:])

                # =========================================================
                # Stages G+H per tile: h3 -> k3 (in place) -> out -> DMA
                # =========================================================
                for b in range(BL):
                    for g in range(G):
                        # G (DVE): k3 = round(clip((h3-mn3)/s3)) via u8 conv
                        ht = h3[(g, b)]
                        nc.vector.tensor_scalar(ht[:], ht[:],
                                                invs3_bc[:, 0:1],
                                                bias3_bc[:, 0:1],
                                                op0=AL.mult, op1=AL.add)
                        kt3 = kpool.tile([P, IMG], u8, name=f'k3_{g}_{b}',
                                         tag='k8')
                        nc.vector.tensor_scalar(kt3[:], ht[:], 1.0, None,
                                                op0=AL.mult)
                        # H (ACT): out = relu(cA3*k3 + cB3)
                        ot = img.tile([P, IMG], f32, name=f'out_{g}_{b}',
                                      tag='img')
                        nc.scalar.activation(ot[:], kt3[:], AF.Relu,
                                             bias=cB3[:, g:g + 1],
                                             scale=cA3[:, g:g + 1])
                        nc.sync.dma_start(
                            out_d[b, g * P:(g + 1) * P].rearrange(
                                'c h w -> c (h w)'), ot[:])

    nc.compile()
    return nc


def _host_consts(dw_w, dw_b, bn1_w, bn1_b, pw_w, bn2_w, bn2_b):
    qdw = _host_quant(dw_w).reshape(256, 9)
    qdb = _host_quant(dw_b)
    qpw = _host_quant(pw_w).reshape(256, 256)
    qbn1w = _host_quant(bn1_w)
    qbn2w = _host_quant(bn2_w)
    wsum = qdw.sum(axis=1, dtype=np.float32)
    pwsum = qpw.sum(axis=1, dtype=np.float32)
    # lhsT layout: pwT[kg, cin, (coutg*128 + cout)] = qpw[cout_full, kg*128+cin]
    pwT = np.ascontiguousarray(
        qpw.T.reshape(G, P, 256)).astype(np.float32)
    consts = {
        'ident': np.eye(P, dtype=np.float32),
        'qdw': np.ascontiguousarray(qdw.reshape(G, P, 9)),
        'wsum': wsum.reshape(G, P).copy(),
        'qdb': qdb.reshape(G, P).copy(),
        'qbn1w': qbn1w.reshape(G, P).copy(),
        'bn1b': np.asarray(bn1_b, np.float32).reshape(G, P).copy(),
        'qbn2w': qbn2w.reshape(G, P).copy(),
        'bn2b': np.asarray(bn2_b, np.float32).reshape(G, P).copy(),
        'pwsum': pwsum.reshape(G, P).copy(),
        'pwT': pwT,
    }
    return consts


def make_in_maps(x, dw_w, dw_b, bn1_w, bn1_b, pw_w, bn2_w, bn2_b):
    x = np.asarray(x, np.float32)
    consts = _host_consts(dw_w, dw_b, bn1_w, bn1_b, pw_w, bn2_w, bn2_b)
    in_maps = []
    for c in range(NCORES):
        m = dict(consts)
        m['x'] = np.ascontiguousarray(x[c * BL:(c + 1) * BL])
        in_maps.append(m)
    return in_maps


def get_program(limit=7):
    if limit not in _PROGRAM_CACHE:
        _PROGRAM_CACHE[limit] = build_program(limit)
    return _PROGRAM_CACHE[limit]


def kernel(**inputs):
    from concourse.bass_utils import run_bass_kernel_spmd
    nc = get_program()
    in_maps = make_in_maps(**inputs)
    res = run_bass_kernel_spmd(nc, in_maps, core_ids=list(range(NCORES)))
    out = np.concatenate([res.results[i]['out'] for i in range(NCORES)],
                         axis=0)
    return out.astype(np.float32)


# revision 21
# speedup vs baseline: 1.4441x; 1.4441x over previous
"""Trainium2 Bass kernel for nn_DepthwiseSeparableFusedConv2d.

Self-contained: takes FULL inputs (x [32,256,56,56] + weights), returns FULL
output [32,256,56,56].  Data-parallel over batch across 8 NeuronCores; the
QuantMeasure / RangeBN global statistics are synchronized with small
AllGather collectives.

v2 design (per core: 4 batches, channels on partitions in 2 groups of 128):
  A:  load x contiguously, per-(channel,sample) min/max
  AG1 (per-sample min/max) -> x quant params;  diag weights scaled by s_x
  B:  quantize x -> integer k (fp32 values) written into padded tiles,
      borders = -mn/s
  C:  depthwise 3x3 conv as 9 accumulating diagonal-weight float32r matmuls;
      ACT evicts PSUM (+bias fold + channel-sum accum); DVE min/max of h1
  AG2 (per-(c,sample) h1 min/max + sums) -> qm(h1) params, RangeBN1 chunk
      stats, BN1 mean (mean over quantized h1 ~= mean h1), analytic qm(h2)
      bounds -- no extra pass, no second collective
  D:  quantize h1 -> u8 k1
  E:  k2 = round(clip(affine(k1))) as fp32 integers (BN1+requant fused)
  F:  pointwise conv k2 @ (qpw*s2) in float32r; ACT evict (+sums);
      DVE min/max of h3; h3 stays in SBUF (no DRAM spill)
  AG5 -> qm(h3) params + RangeBN2 stats + BN2 mean
  G:  requantize h3 -> k3 (in place);  H: out = relu(BN2(k3)) -> DMA out
"""

import math
import numpy as np

# ---------------------------------------------------------------- constants
P = 128
G = 2                 # channel groups (256 = 2*128)
B_FULL = 32
BL = 4                # batches per core
NCORES = 8
HH = 56
IMG = HH * HH         # 3136
PADW = 58
PADI = PADW * PADW    # 3364
MAGIC = 12582912.0    # 1.5 * 2**23  (fp32 round-to-nearest-even trick)
QMAX = 255.0
N_TOT = B_FULL * IMG  # 100352
NCHUNKS = 16
EPS = 1e-5
_N_CHUNK_EL = B_FULL * IMG // NCHUNKS
SCALE_FIX = float((0.5 * 0.35) * (1 + (math.pi * math.log(4)) ** 0.5)
                  / ((2 * math.log(_N_CHUNK_EL)) ** 0.5))

_PROGRAM_CACHE = {}


def _host_quant(w):
    w = np.asarray(w, np.float32)
    mn = w.min()
    mx = w.max()
    scale = np.maximum(((mx - mn) / np.float32(QMAX)).astype(np.float32),
                       np.float32(1e-8))
    t = np.clip((w - mn) / scale, np.float32(0.0), np.float32(QMAX)).astype(np.float32)
    return (np.round(t) * scale + mn).astype(np.float32)


def build_program(limit=7):  # limit unused in full build
    import concourse.bacc as bacc
    import concourse.mybir as mybir
    import concourse.tile as tile

    f32 = mybir.dt.float32
    f32r = mybir.dt.float32r
    bf16 = mybir.dt.bfloat16
    u8 = mybir.dt.uint8
    AL = mybir.AluOpType
    AF = mybir.ActivationFunctionType
    AX = mybir.AxisListType

    nc = bacc.Bacc('TRN2', target_bir_lowering=False, debug=False,
                   num_devices=NCORES)

    # ------------------------------------------------ external tensors
    x_in = nc.dram_tensor('x', [BL, 256, HH, HH], f32, kind='ExternalInput')
    ident_in = nc.dram_tensor('ident', [P, P], f32, kind='ExternalInput')
    qdw_in = nc.dram_tensor('qdw', [G, P, 9], f32, kind='ExternalInput')
    wsum_in = nc.dram_tensor('wsum', [G, P], f32, kind='ExternalInput')
    qdb_in = nc.dram_tensor('qdb', [G, P], f32, kind='ExternalInput')
    qbn1w_in = nc.dram_tensor('qbn1w', [G, P], f32, kind='ExternalInput')
    bn1b_in = nc.dram_tensor('bn1b', [G, P], f32, kind='ExternalInput')
    qbn2w_in = nc.dram_tensor('qbn2w', [G, P], f32, kind='ExternalInput')
    bn2b_in = nc.dram_tensor('bn2b', [G, P], f32, kind='ExternalInput')
    pwsum_in = nc.dram_tensor('pwsum', [G, P], f32, kind='ExternalInput')
    # pwT[kg, cin(128), (coutg, cout)] : lhsT layout, already transposed
    pwT_in = nc.dram_tensor('pwT', [G, P, 256], f32, kind='ExternalInput')
    out_d = nc.dram_tensor('out', [BL, 256, HH, HH], f32, kind='ExternalOutput')

    rg = [list(range(NCORES))]

    # pointwise free-dim chunks: 7 x 448 (all >= 256 so f32r runs 1 cyc/col)
    PWC = 448
    PW_CHUNKS = [(i * PWC, PWC) for i in range(7)]

    with tile.TileContext(nc) as tc:
        with (
            tc.tile_pool(name='perm', bufs=1) as perm,
            tc.tile_pool(name='kpool', bufs=6) as kpool,
            tc.tile_pool(name='k2p', bufs=3) as k2p,
            tc.tile_pool(name='dram', bufs=1, space='DRAM') as dpool,
            tc.tile_pool(name='tp', bufs=2, space='PSUM') as tpp,
        ):
            # ------------------------------------------------ constants
            ident = perm.tile([P, P], f32, name='identsb')
            nc.sync.dma_start(ident[:], ident_in[:])
            # warmup collective: absorbs the CC cold-start latency
            # while phase A runs
            wu = perm.tile([1, 2], f32, name='wu')
            nc.vector.memset(wu[:], 0.0)
            ag0_in = dpool.tile([2], f32, name='ag0_in')
            ag0_out = dpool.tile([NCORES * 2], f32, name='ag0_out')
            nc.sync.dma_start(ag0_in[None, :], wu[:])
            nc.gpsimd.collective_compute(
                'AllGather', AL.bypass, replica_groups=rg,
                ins=[ag0_in[:].opt()], outs=[ag0_out[:].opt()])
            qdw = perm.tile([P, G, 9], f32, name='qdwsb')
            nc.sync.dma_start(qdw[:], qdw_in.rearrange('g c t -> c g t'))

            def load_gp(t_in, nm):
                t = perm.tile([P, G], f32, name=nm)
                nc.sync.dma_start(t[:], t_in.rearrange('g c -> c g'))
                return t
            wsum_t = load_gp(wsum_in, 'wsumsb')
            qdb_t = load_gp(qdb_in, 'qdbsb')
            qbn1w_t = load_gp(qbn1w_in, 'qbn1wsb')
            bn1b_t = load_gp(bn1b_in, 'bn1bsb')
            qbn2w_t = load_gp(qbn2w_in, 'qbn2wsb')
            bn2b_t = load_gp(bn2b_in, 'bn2bsb')
            pwsum_t = load_gp(pwsum_in, 'pwsumsb')
            pwT = perm.tile([P, G, 256], f32, name='pwTsb')
            nc.sync.dma_start(pwT[:], pwT_in[:].rearrange('g c m -> c g m'))

            # diag weight matrices (filled after AG1: qdw * s_x folded in)
            diagt = perm.tile([P, G * 9 * P], f32r, name='diagt')

            # ------------------------------------------------ stat tiles
            # per-(c,b): [min(BL) | max(BL) | sum(BL)]
            xstat = [perm.tile([P, 2 * BL], f32, name=f'xstat{g}')
                     for g in range(G)]
            h1stat = [perm.tile([P, 3 * BL], f32, name=f'h1stat{g}')
                      for g in range(G)]
            h3stat = [perm.tile([P, 3 * BL], f32, name=f'h3stat{g}')
                      for g in range(G)]
            hsum8 = [perm.tile([P, BL, 8], f32, name=f'hsum8_{g}')
                     for g in range(G)]
            Ag = [perm.tile([P, 3, NCORES, 2, 2], f32, name=f'Ag{g}')
                  for g in range(G)]
            cstat = [perm.tile([P, 2, NCORES * 2], f32, name=f'cstat{g}')
                     for g in range(G)]

            def sc1(nm):
                return perm.tile([1, 1], f32, name=nm)

            def bc1(nm):
                return perm.tile([P, 1], f32, name=nm)

            # scalar math helper: from sum-of-mins/sum-of-maxes -> quant params
            def quant_params(mnsum, mxsum, tag):
                mn = sc1(f'mn_{tag}')
                mx = sc1(f'mx_{tag}')
                nc.vector.tensor_scalar(mn[:], mnsum[:], 1.0 / B_FULL, None, op0=AL.mult)
                nc.vector.tensor_scalar(mx[:], mxsum[:], 1.0 / B_FULL, None, op0=AL.mult)
                d = sc1(f'd_{tag}')
                nc.vector.tensor_sub(d[:], mx[:], mn[:])
                s = sc1(f's_{tag}')
                nc.vector.tensor_scalar(s[:], d[:], 1.0 / QMAX, 1e-8,
                                        op0=AL.mult, op1=AL.max)
                inv_s = sc1(f'invs_{tag}')
                nc.vector.reciprocal(inv_s[:], s[:])
                negmn = sc1(f'negmn_{tag}')
                nc.vector.tensor_scalar(negmn[:], mn[:], -1.0, None, op0=AL.mult)
                bias = sc1(f'bias_{tag}')
                nc.vector.tensor_mul(bias[:], negmn[:], inv_s[:])
                return {'mn': mn, 'mx': mx, 's': s, 'inv_s': inv_s,
                        'negmn': negmn, 'bias': bias}

            def bcast(src, nm):
                t = bc1(nm)
                nc.gpsimd.partition_broadcast(t[:], src[:])
                return t

            # quantize chain on a small [P, F] stat tile (value form k, fp32)
            def qchain_small(ap, inv_s_bc, bias_bc):
                nc.scalar.activation(ap, ap, AF.Relu, bias=bias_bc[:, 0:1],
                                     scale=inv_s_bc[:, 0:1])
                nc.vector.tensor_scalar(ap, ap, QMAX, MAGIC, op0=AL.min, op1=AL.add)
                nc.vector.tensor_scalar(ap, ap, MAGIC, None, op0=AL.subtract)

            def minmax_tree(nm, src_ap, mn_ap, mx_ap):
                nc.vector.tensor_reduce(mn_ap, src_ap, axis=AX.X, op=AL.min)
                nc.vector.tensor_reduce(mx_ap, src_ap, axis=AX.X, op=AL.max)

            # =================================================================
            # Stage A: load x (contiguous) + x stats
            # =================================================================
            with tc.tile_pool(name='img', bufs=11) as img:
                raw = {}
                for b in range(BL):
                    for g in range(G):
                        t = img.tile([P, IMG], f32, name=f'raw{g}_{b}',
                                     tag='img')
                        raw[(g, b)] = t
                        nc.sync.dma_start(
                            t[:].rearrange('p (h w) -> p h w', h=HH),
                            x_in[b, g * P:(g + 1) * P])
                        minmax_tree(f'xs{g}_{b}', t[:],
                                    xstat[g][:, b:b + 1],
                                    xstat[g][:, BL + b:BL + b + 1])

                # --- AG1: per-sample min/max (8 floats per core) ---
                tmin = tpp.tile([BL, G * P], f32, name='tmin1', tag='tp')
                tmax = tpp.tile([BL, G * P], f32, name='tmax1', tag='tp')
                for g in range(G):
                    nc.tensor.transpose(tmin[:, g * P:(g + 1) * P],
                                        xstat[g][:, 0:BL], ident[:])
                    nc.tensor.transpose(tmax[:, g * P:(g + 1) * P],
                                        xstat[g][:, BL:2 * BL], ident[:])
                ab1 = perm.tile([BL, 2], f32, name='ab1')
                nc.vector.tensor_reduce(ab1[:, 0:1], tmin[:], axis=AX.X, op=AL.min)
                nc.vector.tensor_reduce(ab1[:, 1:2], tmax[:], axis=AX.X, op=AL.max)

                ag1_in = dpool.tile([BL * 2], f32, name='ag1_in')
                ag1_out = dpool.tile([NCORES * BL * 2], f32, name='ag1_out')
                nc.sync.dma_start(ag1_in.rearrange('(b s) -> b s', s=2), ab1[:])
                nc.gpsimd.collective_compute(
                    'AllGather', AL.bypass, replica_groups=rg,
                    ins=[ag1_in[:].opt()], outs=[ag1_out[:].opt()])
                agb1 = perm.tile([1, NCORES * BL * 2], f32, name='agb1')
                nc.sync.dma_start(agb1[:], ag1_out[None, :])
                v1 = agb1.rearrange('p (cb s) -> p s cb', s=2)
                mnsum_x = sc1('mnsum_x')
                mxsum_x = sc1('mxsum_x')
                nc.vector.tensor_reduce(mnsum_x[:], v1[:, 0, :], axis=AX.X, op=AL.add)
                nc.vector.tensor_reduce(mxsum_x[:], v1[:, 1, :], axis=AX.X, op=AL.add)
                qx = quant_params(mnsum_x, mxsum_x, 'x')
                invsx_bc = bcast(qx['inv_s'], 'invsx_bc')
                biasx_bc = bcast(qx['bias'], 'biasx_bc')
                sx_bc = bcast(qx['s'], 'sx_bc')
                # centered k' = k - 128 halves |k| so f32r operand rounding
                # of the products shrinks ~5x.  border k' = -mn/s - 128;
                # wsum correction uses mn' = mn + 128*s.
                nmos128 = sc1('nmos128')
                nc.vector.tensor_scalar(nmos128[:], qx['bias'][:], -128.0,
                                        None, op0=AL.add)
                nmos_bc = bcast(nmos128, 'nmos_bc')
                mnp = sc1('mnp')
                nc.vector.tensor_scalar(mnp[:], qx['s'][:], 128.0,
                                        qx['mn'][:, 0:1],
                                        op0=AL.mult, op1=AL.add)
                mnx_bc = bcast(mnp, 'mnx_bc')

                # diag weights: ident * (qdw[c,g,t] * s_x)
                for g in range(G):
                    for t in range(9):
                        i = g * 9 + t
                        nc.vector.tensor_scalar(
                            diagt[:, i * P:(i + 1) * P], ident[:],
                            qdw[:, g, t:t + 1], sx_bc[:, 0:1],
                            op0=AL.mult, op1=AL.mult)

                # const1 = qdb + mn_x * wsum  (per channel)
                const1 = perm.tile([P, G], f32, name='const1')
                for g in range(G):
                    nc.vector.scalar_tensor_tensor(
                        const1[:, g:g + 1], wsum_t[:, g:g + 1], mnx_bc[:, 0:1],
                        qdb_t[:, g:g + 1], op0=AL.mult, op1=AL.add)

                # constant border strip: value -mn/s everywhere
                bord = perm.tile([P, PADW], f32r, name='bord')
                nc.vector.tensor_scalar(bord[:], ident[:, 0:PADW], 0.0,
                                        nmos_bc[:, 0:1],
                                        op0=AL.mult, op1=AL.add)

                # =========================================================
                # Stage B+C per tile: quantize into padded tile, conv, evict
                # =========================================================
                h1 = {}
                with tc.tile_pool(name='cv', bufs=6, space='PSUM') as cvp:
                    for b in range(BL):
                        for g in range(G):
                            rt = raw[(g, b)]
                            # B: k = round(clip((x-mn)/s)) via saturating
                            # u8 convert (RNE), then center to k-128 (f32r)
                            ku = kpool.tile([P, IMG], u8, name=f'kx{g}_{b}',
                                            tag='k8')
                            nc.scalar.activation(ku[:], rt[:], AF.Relu,
                                                 bias=biasx_bc[:, 0:1],
                                                 scale=invsx_bc[:, 0:1])
                            xp = img.tile([P, PADI], f32r, name=f'xp{g}_{b}',
                                          tag='img')
                            v = xp.rearrange('p (h w) -> p h w', h=PADW)
                            nc.vector.tensor_scalar(
                                v[:, 1:57, 1:57],
                                ku[:].rearrange('p (h w) -> p h w', h=HH),
                                128.0, None, op0=AL.subtract)
                            # borders := -mn/s (cancels wsum correction)
                            for bap, bw in ((v[:, 0, :], PADW),
                                            (v[:, 57, :], PADW),
                                            (v[:, 1:57, 0], HH),
                                            (v[:, 1:57, 57], HH)):
                                nc.vector.tensor_scalar(bap, bord[:, 0:bw],
                                                        1.0, None, op0=AL.mult)
                            # C: depthwise conv via diag f32r matmuls
                            src = v
                            h1t = img.tile([P, IMG], f32, name=f'h1_{g}_{b}',
                                           tag='img')
                            h1[(g, b)] = h1t
                            for half in range(2):
                                pst = [cvp.tile([P, 392], f32,
                                                name=f'cv{g}{b}{half}{rb}',
                                                tag='cv')
                                       for rb in range(4)]
                                for t in range(9):
                                    di, dj = t // 3, t % 3
                                    lhs = diagt[:, (g * 9 + t) * P:(g * 9 + t + 1) * P]
                                    for rb in range(4):
                                        r0 = half * 28 + rb * 7
                                        rhs = src[:, r0 + di:r0 + di + 7,
                                                  dj:dj + 56]
                                        nc.tensor.matmul(
                                            pst[rb][:], lhs, rhs,
                                            start=(t == 0), stop=(t == 8))
                                for rb in range(4):
                                    r0 = half * 28 + rb * 7
                                    j = half * 4 + rb
                                    nc.scalar.activation(
                                        h1t[:, r0 * HH:(r0 + 7) * HH],
                                        pst[rb][:], AF.Identity,
                                        bias=const1[:, g:g + 1], scale=1.0,
                                        accum_out=hsum8[g][:, b, j:j + 1])
                            minmax_tree(f'h1s{g}_{b}', h1t[:],
                                        h1stat[g][:, b:b + 1],
                                        h1stat[g][:, BL + b:BL + b + 1])
                            nc.vector.tensor_reduce(
                                h1stat[g][:, 2 * BL + b:2 * BL + b + 1],
                                hsum8[g][:, b], axis=AX.X, op=AL.add)

                # =========================================================
                # AG2: per-(channel,sample) h1 min/max/sum
                # =========================================================
                ag2_in = dpool.tile([G * P * 3 * BL], f32, name='ag2_in')
                ag2_out = dpool.tile([NCORES * G * P * 3 * BL], f32, name='ag2_out')
                v2i = ag2_in.rearrange('(g c f) -> g c f', g=G, c=P)
                for g in range(G):
                    nc.sync.dma_start(v2i[g], h1stat[g][:])
                nc.gpsimd.collective_compute(
                    'AllGather', AL.bypass, replica_groups=rg,
                    ins=[ag2_in[:].opt()], outs=[ag2_out[:].opt()])
                v2o = ag2_out.rearrange(
                    '(core g c s q b) -> g c s core q b',
                    core=NCORES, g=G, c=P, s=3, q=2)
                for g in range(G):
                    for s in range(3):
                        nc.sync.dma_start(Ag[g][:, s], v2o[g][:, s])

                # chunk stats (pair of batches within a core)
                for g in range(G):
                    nc.vector.tensor_reduce(
                        cstat[g][:, 0, :], Ag[g][:, 0], axis=AX.X, op=AL.min)
                    nc.vector.tensor_reduce(
                        cstat[g][:, 1, :], Ag[g][:, 1], axis=AX.X, op=AL.max)

                # per-sample min/max over all 256 channels -> qm() params
                def sample_params(stattiles, tag):
                    tmn = tpp.tile([B_FULL, G * P], f32, name=f'tmn_{tag}', tag='tp')
                    tmx = tpp.tile([B_FULL, G * P], f32, name=f'tmx_{tag}', tag='tp')
                    for g in range(G):
                        flat = stattiles[g].rearrange('p s core q b -> p (s core q b)')
                        nc.tensor.transpose(tmn[:, g * P:(g + 1) * P],
                                            flat[:, 0:B_FULL], ident[:])
                        nc.tensor.transpose(tmx[:, g * P:(g + 1) * P],
                                            flat[:, B_FULL:2 * B_FULL], ident[:])
                    pm = perm.tile([B_FULL, 2], f32, name=f'pm_{tag}')
                    nc.vector.tensor_reduce(pm[:, 0:1], tmn[:], axis=AX.X, op=AL.min)
                    nc.vector.tensor_reduce(pm[:, 1:2], tmx[:], axis=AX.X, op=AL.max)
                    ta = tpp.tile([1, B_FULL], f32, name=f'ta_{tag}', tag='tp')
                    tb = tpp.tile([1, B_FULL], f32, name=f'tb_{tag}', tag='tp')
                    nc.tensor.transpose(ta[:], pm[:, 0:1], ident[0:B_FULL, 0:B_FULL])
                    nc.tensor.transpose(tb[:], pm[:, 1:2], ident[0:B_FULL, 0:B_FULL])
                    mnsum = sc1(f'mnsum_{tag}')
                    mxsum = sc1(f'mxsum_{tag}')
                    nc.vector.tensor_reduce(mnsum[:], ta[:], axis=AX.X, op=AL.add)
                    nc.vector.tensor_reduce(mxsum[:], tb[:], axis=AX.X, op=AL.add)
                    return quant_params(mnsum, mxsum, tag)

                q1 = sample_params(Ag, 'h1')
                invs1_bc = bcast(q1['inv_s'], 'invs1_bc')
                bias1_bc = bcast(q1['bias'], 'bias1_bc')
                s1_bc = bcast(q1['s'], 's1_bc')
                mn1_bc = bcast(q1['mn'], 'mn1_bc')

                # RangeBN scale from chunk stats
                def rangebn_scale(cstat_g, invs_bc, bias_bc, s_bc, mn_bc, tag):
                    scpk = perm.tile([P, G], f32, name=f'scpk_{tag}')
                    for g in range(G):
                        c = cstat_g[g].rearrange('p s f -> p (s f)')
                        qchain_small(c[:, :], invs_bc, bias_bc)
                        # now c holds integer k; mean over 16 chunks, value form
                        mm = perm.tile([P, 2], f32, name=f'mm_{tag}{g}')
                        nc.vector.tensor_reduce(
                            mm[:], cstat_g[g][:], axis=AX.X, op=AL.add)
                        # mm = (sum k)/16 * s + mn
                        nc.vector.tensor_scalar(mm[:], mm[:], 1.0 / NCHUNKS,
                                                s_bc[:, 0:1],
                                                op0=AL.mult, op1=AL.mult)
                        nc.vector.tensor_scalar(mm[:], mm[:], mn_bc[:, 0:1],
                                                None, op0=AL.add)
                        d = perm.tile([P, 1], f32, name=f'dmm_{tag}{g}')
                        nc.vector.tensor_sub(d[:], mm[:, 1:2], mm[:, 0:1])
                        nc.vector.tensor_scalar(d[:], d[:], SCALE_FIX, EPS,
                                                op0=AL.mult, op1=AL.add)
                        nc.vector.reciprocal(scpk[:, g:g + 1], d[:])
                    # quantize scale over all 256 channels
                    tq = tpp.tile([1, G * P], f32, name=f'tq_{tag}', tag='tp')
                    for g in range(G):
                        nc.tensor.transpose(tq[:, g * P:(g + 1) * P],
                                            scpk[:, g:g + 1], ident[:])
                    smn = sc1(f'smn_{tag}')
                    smx = sc1(f'smx_{tag}')
                    nc.vector.tensor_reduce(smn[:], tq[:], axis=AX.X, op=AL.min)
                    nc.vector.tensor_reduce(smx[:], tq[:], axis=AX.X, op=AL.max)
                    dd = sc1(f'sd_{tag}')
                    nc.vector.tensor_sub(dd[:], smx[:], smn[:])
                    ss = sc1(f'ss_{tag}')
                    nc.vector.tensor_scalar(ss[:], dd[:], 1.0 / QMAX, 1e-8,
                                            op0=AL.mult, op1=AL.max)
                    invss = sc1(f'invss_{tag}')
                    nc.vector.reciprocal(invss[:], ss[:])
                    negsmn = sc1(f'negsmn_{tag}')
                    nc.vector.tensor_scalar(negsmn[:], smn[:], -1.0, None, op0=AL.mult)
                    bss = sc1(f'bss_{tag}')
                    nc.vector.tensor_mul(bss[:], negsmn[:], invss[:])
                    invss_bc = bcast(invss, f'invss_bc_{tag}')
                    bss_bc = bcast(bss, f'bss_bc_{tag}')
                    ss_bc = bcast(ss, f'ss_bc_{tag}')
                    smn_bc = bcast(smn, f'smn_bc_{tag}')
                    qchain_small(scpk[:, :], invss_bc, bss_bc)
                    nc.vector.tensor_scalar(scpk[:], scpk[:], ss_bc[:, 0:1],
                                            None, op0=AL.mult)
                    nc.vector.tensor_scalar(scpk[:], scpk[:], smn_bc[:, 0:1],
                                            None, op0=AL.add)
                    return scpk

                qscale1 = rangebn_scale(cstat, invs1_bc, bias1_bc, s1_bc,
                                        mn1_bc, 'bn1')
                A1 = perm.tile([P, G], f32, name='A1')
                nc.vector.tensor_mul(A1[:], qscale1[:], qbn1w_t[:])
                cA1 = perm.tile([P, G], f32, name='cA1')
                nc.vector.tensor_scalar(cA1[:], A1[:], s1_bc[:, 0:1], None,
                                        op0=AL.mult)

                # mean1 = (sum over cores+batches of h1 sums) / N_TOT
                mean1 = perm.tile([P, G], f32, name='mean1')
                for g in range(G):
                    fsum = Ag[g].rearrange('p s core q b -> p s (core q b)')
                    nc.vector.tensor_reduce(mean1[:, g:g + 1],
                                            fsum[:, 2, :], axis=AX.X, op=AL.add)
                nc.vector.tensor_scalar(mean1[:], mean1[:], 1.0 / N_TOT, None,
                                        op0=AL.mult)
                cB1 = perm.tile([P, G], f32, name='cB1')
                nc.vector.tensor_scalar(cB1[:], mean1[:], -1.0,
                                        mn1_bc[:, 0:1], op0=AL.mult, op1=AL.add)
                # cB1 currently = (mn1 - mean1); multiply by A1, add bn1b
                nc.vector.tensor_mul(cB1[:], cB1[:], A1[:])
                nc.vector.tensor_add(cB1[:], cB1[:], bn1b_t[:])

                # analytic qm(h2) bounds from Ag (monotone: cA1 >= 0)
                for g in range(G):
                    flat = Ag[g].rearrange('p s core q b -> p (s core q b)')
                    ext = flat[:, 0:2 * B_FULL]
                    qchain_small(ext, invs1_bc, bias1_bc)
                    nc.scalar.activation(ext, ext,
                                         AF.Relu, bias=cB1[:, g:g + 1],
                                         scale=cA1[:, g:g + 1])
                q2 = sample_params(Ag, 'h2')
                invs2_bc = bcast(q2['inv_s'], 'invs2_bc')
                bias2_bc = bcast(q2['bias'], 'bias2_bc')
                s2_bc = bcast(q2['s'], 's2_bc')
                mn2_bc = bcast(q2['mn'], 'mn2_bc')

                # E-stage fused affine: k2 = round(clip(aE*k1 + bE, 0, 255))
                aE = perm.tile([P, G], f32, name='aE')
                bE = perm.tile([P, G], f32, name='bE')
                nc.vector.tensor_scalar(aE[:], cA1[:], invs2_bc[:, 0:1], None,
                                        op0=AL.mult)
                nc.vector.tensor_scalar(bE[:], cB1[:], mn2_bc[:, 0:1],
                                        invs2_bc[:, 0:1],
                                        op0=AL.subtract, op1=AL.mult)

                # scaled pointwise weights (bf16: k2 integers exact, weight
                # rounding in the 256-way contraction is harmless) + const3
                pwTs = perm.tile([P, G, 256], bf16, name='pwTs')
                nc.vector.tensor_scalar(pwTs[:], pwT[:], s2_bc[:, 0:1], None,
                                        op0=AL.mult)
                const3 = perm.tile([P, G], f32, name='const3')
                nc.vector.tensor_scalar(const3[:], pwsum_t[:], mn2_bc[:, 0:1],
                                        None, op0=AL.mult)

                # =========================================================
                # Stages D+E+F per batch: h1 -> k1(u8) -> k2(u8) -> bf16
                # -> h3 (SBUF); Sum(h3) accumulated during eviction
                # =========================================================
                h3 = {}
                with tc.tile_pool(name='pw', bufs=6, space='PSUM') as pwp:
                    for b in range(BL):
                        k2b = {}
                        for g in range(G):
                            # D (DVE): k1 = round(clip((h1-mn1)/s1)) --
                            # the u8 convert does RNE + [0,255] saturation
                            ht = h1[(g, b)]
                            nc.vector.tensor_scalar(ht[:], ht[:],
                                                    invs1_bc[:, 0:1],
                                                    bias1_bc[:, 0:1],
                                                    op0=AL.mult, op1=AL.add)
                            kt = kpool.tile([P, IMG], u8, name=f'k1_{g}_{b}',
                                            tag='k8')
                            nc.vector.tensor_scalar(kt[:], ht[:], 1.0, None,
                                                    op0=AL.mult)
                            # E: k2 = round(clip(aE*k1 + bE)), in place u8->u8
                            nc.scalar.activation(kt[:], kt[:], AF.Relu,
                                                 bias=bE[:, g:g + 1],
                                                 scale=aE[:, g:g + 1])
                            # convert to bf16 for the pointwise matmuls
                            k2t = k2p.tile([P, IMG], bf16, name=f'k2_{g}_{b}',
                                           tag='k2')
                            nc.vector.tensor_scalar(k2t[:], kt[:], 1.0, None,
                                                    op0=AL.mult)
                            k2b[g] = k2t
                        # F: pointwise conv for this batch (bf16)
                        for cg in range(G):
                            h3t = img.tile([P, IMG], f32, name=f'h3_{cg}_{b}',
                                           tag='img')
                            h3[(cg, b)] = h3t
                            for blk in (PW_CHUNKS[0:4], PW_CHUNKS[4:7]):
                                pst = {}
                                for (c0, nn) in blk:
                                    pst[c0] = pwp.tile([P, PWC], f32,
                                                       name=f'pw{cg}{b}{c0}',
                                                       tag='pw')
                                for kg in range(G):
                                    lhs = pwTs[:, kg, cg * P:(cg + 1) * P]
                                    for (c0, nn) in blk:
                                        nc.tensor.matmul(
                                            pst[c0][:, 0:nn], lhs,
                                            k2b[kg][:, c0:c0 + nn],
                                            start=(kg == 0), stop=(kg == 1))
                                for ji, (c0, nn) in enumerate(blk):
                                    j = (0 if c0 < 4 * PWC else 4) + ji
                                    nc.scalar.activation(
                                        h3t[:, c0:c0 + nn], pst[c0][:, 0:nn],
                                        AF.Identity, bias=const3[:, cg:cg + 1],
                                        scale=1.0,
                                        accum_out=hsum8[cg][:, b, j:j + 1])
                            minmax_tree(f'h3s{cg}_{b}', h3t[:],
                                        h3stat[cg][:, b:b + 1],
                                        h3stat[cg][:, BL + b:BL + b + 1])
                            nc.vector.tensor_reduce(
                                h3stat[cg][:, 2 * BL + b:2 * BL + b + 1],
                                hsum8[cg][:, b, 0:7], axis=AX.X, op=AL.add)

                # =========================================================
                # AG5 + RangeBN2 stats
                # =========================================================
                ag5_in = dpool.tile([G * P * 3 * BL], f32, name='ag5_in')
                ag5_out = dpool.tile([NCORES * G * P * 3 * BL], f32,
                                     name='ag5_out')
                v5i = ag5_in.rearrange('(g c f) -> g c f', g=G, c=P)
                for g in range(G):
                    nc.sync.dma_start(v5i[g], h3stat[g][:])
                nc.gpsimd.collective_compute(
                    'AllGather', AL.bypass, replica_groups=rg,
                    ins=[ag5_in[:].opt()], outs=[ag5_out[:].opt()])
                v5o = ag5_out.rearrange(
                    '(core g c s q b) -> g c s core q b',
                    core=NCORES, g=G, c=P, s=3, q=2)
                for g in range(G):
                    for s in range(3):
                        nc.sync.dma_start(Ag[g][:, s], v5o[g][:, s])
                for g in range(G):
                    nc.vector.tensor_reduce(
                        cstat[g][:, 0, :], Ag[g][:, 0], axis=AX.X, op=AL.min)
                    nc.vector.tensor_reduce(
                        cstat[g][:, 1, :], Ag[g][:, 1], axis=AX.X, op=AL.max)
                q3 = sample_params(Ag, 'h3')
                invs3_bc = bcast(q3['inv_s'], 'invs3_bc')
                bias3_bc = bcast(q3['bias'], 'bias3_bc')
                s3_bc = bcast(q3['s'], 's3_bc')
                mn3_bc = bcast(q3['mn'], 'mn3_bc')
                qscale3 = rangebn_scale(cstat, invs3_bc, bias3_bc, s3_bc,
                                        mn3_bc, 'bn2')
                A3 = perm.tile([P, G], f32, name='A3')
                nc.vector.tensor_mul(A3[:], qscale3[:], qbn2w_t[:])
                cA3 = perm.tile([P, G], f32, name='cA3')
                nc.vector.tensor_scalar(cA3[:], A3[:], s3_bc[:, 0:1], None,
                                        op0=AL.mult)
                mean3 = perm.tile([P, G], f32, name='mean3')
                for g in range(G):
                    fsum = Ag[g].rearrange('p s core q b -> p s (core q b)')
                    nc.vector.tensor_reduce(mean3[:, g:g + 1],
                                            fsum[:, 2, :], axis=AX.X, op=AL.add)
                nc.vector.tensor_scalar(mean3[:], mean3[:], 1.0 / N_TOT, None,
                                        op0=AL.mult)
                cB3 = perm.tile([P, G], f32, name='cB3')
                nc.vector.tensor_scalar(cB3[:], mean3[:], -1.0,
                                        mn3_bc[:, 0:1], op0=AL.mult, op1=AL.add)
                nc.vector.tensor_mul(cB3[:], cB3[:], A3[:])
                nc.vector.tensor_add(cB3[:], cB3[:], bn2b_t[:])

                # =========================================================
                # Stages G+H per tile: h3 -> k3 (in place) -> out -> DMA
                # =========================================================
                for b in range(BL):
                    for g in range(G):
                        # G (DVE): k3 = round(clip((h3-mn3)/s3)) via u8 conv
                        ht = h3[(g, b)]
                        nc.vector.tensor_scalar(ht[:], ht[:],
                                                invs3_bc[:, 0:1],
                                                bias3_bc[:, 0:1],
                                                op0=AL.mult, op1=AL.add)
                        kt3 = kpool.tile([P, IMG], u8, name=f'k3_{g}_{b}',
                                         tag='k8')
                        nc.vector.tensor_scalar(kt3[:], ht[:], 1.0, None,
                                                op0=AL.mult)
                        # H (ACT): out = relu(cA3*k3 + cB3)
                        ot = img.tile([P, IMG], f32, name=f'out_{g}_{b}',
                                      tag='img')
                        nc.scalar.activation(ot[:], kt3[:], AF.Relu,
                                             bias=cB3[:, g:g + 1],
                                             scale=cA3[:, g:g + 1])
                        nc.sync.dma_start(
                            out_d[b, g * P:(g + 1) * P].rearrange(
                                'c h w -> c (h w)'), ot[:])

    nc.compile()
    return nc


def _host_consts(dw_w, dw_b, bn1_w, bn1_b, pw_w, bn2_w, bn2_b):
    qdw = _host_quant(dw_w).reshape(256, 9)
    qdb = _host_quant(dw_b)
    qpw = _host_quant(pw_w).reshape(256, 256)
    qbn1w = _host_quant(bn1_w)
    qbn2w = _host_quant(bn2_w)
    wsum = qdw.sum(axis=1, dtype=np.float32)
    pwsum = qpw.sum(axis=1, dtype=np.float32)
    # lhsT layout: pwT[kg, cin, (coutg*128 + cout)] = qpw[cout_full, kg*128+cin]
    pwT = np.ascontiguousarray(
        qpw.T.reshape(G, P, 256)).astype(np.float32)
    consts = {
        'ident': np.eye(P, dtype=np.float32),
        'qdw': np.ascontiguousarray(qdw.reshape(G, P, 9)),
        'wsum': wsum.reshape(G, P).copy(),
        'qdb': qdb.reshape(G, P).copy(),
        'qbn1w': qbn1w.reshape(G, P).copy(),
        'bn1b': np.asarray(bn1_b, np.float32).reshape(G, P).copy(),
        'qbn2w': qbn2w.reshape(G, P).copy(),
        'bn2b': np.asarray(bn2_b, np.float32).reshape(G, P).copy(),
        'pwsum': pwsum.reshape(G, P).copy(),
        'pwT': pwT,
    }
    return consts


def make_in_maps(x, dw_w, dw_b, bn1_w, bn1_b, pw_w, bn2_w, bn2_b):
    x = np.asarray(x, np.float32)
    consts = _host_consts(dw_w, dw_b, bn1_w, bn1_b, pw_w, bn2_w, bn2_b)
    in_maps = []
    for c in range(NCORES):
        m = dict(consts)
        m['x'] = np.ascontiguousarray(x[c * BL:(c + 1) * BL])
        in_maps.append(m)
    return in_maps


def get_program(limit=7):
    if limit not in _PROGRAM_CACHE:
        _PROGRAM_CACHE[limit] = build_program(limit)
    return _PROGRAM_CACHE[limit]


def kernel(**inputs):
    from concourse.bass_utils import run_bass_kernel_spmd
    nc = get_program()
    in_maps = make_in_maps(**inputs)
    res = run_bass_kernel_spmd(nc, in_maps, core_ids=list(range(NCORES)))
    out = np.concatenate([res.results[i]['out'] for i in range(NCORES)],
                         axis=0)
    return out.astype(np.float32)


# revision 22
# speedup vs baseline: 1.5972x; 1.1060x over previous
"""Trainium2 Bass kernel for nn_DepthwiseSeparableFusedConv2d.

Self-contained: takes FULL inputs (x [32,256,56,56] + weights), returns FULL
output [32,256,56,56].  Data-parallel over batch across 8 NeuronCores; the
QuantMeasure / RangeBN global statistics are synchronized with small
AllGather collectives.

v2 design (per core: 4 batches, channels on partitions in 2 groups of 128):
  A:  load x contiguously, per-(channel,sample) min/max
  AG1 (per-sample min/max) -> x quant params;  diag weights scaled by s_x
  B:  quantize x -> integer k (fp32 values) written into padded tiles,
      borders = -mn/s
  C:  depthwise 3x3 conv as 9 accumulating diagonal-weight float32r matmuls;
      ACT evicts PSUM (+bias fold + channel-sum accum); DVE min/max of h1
  AG2 (per-(c,sample) h1 min/max + sums) -> qm(h1) params, RangeBN1 chunk
      stats, BN1 mean (mean over quantized h1 ~= mean h1), analytic qm(h2)
      bounds -- no extra pass, no second collective
  D:  quantize h1 -> u8 k1
  E:  k2 = round(clip(affine(k1))) as fp32 integers (BN1+requant fused)
  F:  pointwise conv k2 @ (qpw*s2) in float32r; ACT evict (+sums);
      DVE min/max of h3; h3 stays in SBUF (no DRAM spill)
  AG5 -> qm(h3) params + RangeBN2 stats + BN2 mean
  G:  requantize h3 -> k3 (in place);  H: out = relu(BN2(k3)) -> DMA out
"""

import math
import numpy as np

# ---------------------------------------------------------------- constants
P = 128
G = 2                 # channel groups (256 = 2*128)
B_FULL = 32
BL = 4                # batches per core
NCORES = 8
HH = 56
IMG = HH * HH         # 3136
PADW = 58
PADI = PADW * PADW    # 3364
MAGIC = 12582912.0    # 1.5 * 2**23  (fp32 round-to-nearest-even trick)
QMAX = 255.0
N_TOT = B_FULL * IMG  # 100352
NCHUNKS = 16
EPS = 1e-5
_N_CHUNK_EL = B_FULL * IMG // NCHUNKS
SCALE_FIX = float((0.5 * 0.35) * (1 + (math.pi * math.log(4)) ** 0.5)
                  / ((2 * math.log(_N_CHUNK_EL)) ** 0.5))

_PROGRAM_CACHE = {}


def _host_quant(w):
    w = np.asarray(w, np.float32)
    mn = w.min()
    mx = w.max()
    scale = np.maximum(((mx - mn) / np.float32(QMAX)).astype(np.float32),
                       np.float32(1e-8))
    t = np.clip((w - mn) / scale, np.float32(0.0), np.float32(QMAX)).astype(np.float32)
    return (np.round(t) * scale + mn).astype(np.float32)


def build_program(limit=7):  # limit unused in full build
    import concourse.bacc as bacc
    import concourse.mybir as mybir
    import concourse.tile as tile

    f32 = mybir.dt.float32
    f32r = mybir.dt.float32r
    bf16 = mybir.dt.bfloat16
    u8 = mybir.dt.uint8
    AL = mybir.AluOpType
    AF = mybir.ActivationFunctionType
    AX = mybir.AxisListType

    nc = bacc.Bacc('TRN2', target_bir_lowering=False, debug=False,
                   num_devices=NCORES)

    # ------------------------------------------------ external tensors
    x_in = nc.dram_tensor('x', [BL, 256, HH, HH], f32, kind='ExternalInput')
    ident_in = nc.dram_tensor('ident', [P, P], f32, kind='ExternalInput')
    qdw_in = nc.dram_tensor('qdw', [G, P, 9], f32, kind='ExternalInput')
    wsum_in = nc.dram_tensor('wsum', [G, P], f32, kind='ExternalInput')
    qdb_in = nc.dram_tensor('qdb', [G, P], f32, kind='ExternalInput')
    qbn1w_in = nc.dram_tensor('qbn1w', [G, P], f32, kind='ExternalInput')
    bn1b_in = nc.dram_tensor('bn1b', [G, P], f32, kind='ExternalInput')
    qbn2w_in = nc.dram_tensor('qbn2w', [G, P], f32, kind='ExternalInput')
    bn2b_in = nc.dram_tensor('bn2b', [G, P], f32, kind='ExternalInput')
    pwsum_in = nc.dram_tensor('pwsum', [G, P], f32, kind='ExternalInput')
    # pwT[kg, cin(128), (coutg, cout)] : lhsT layout, already transposed
    pwT_in = nc.dram_tensor('pwT', [G, P, 256], f32, kind='ExternalInput')
    out_d = nc.dram_tensor('out', [BL, 256, HH, HH], f32, kind='ExternalOutput')

    rg = [list(range(NCORES))]

    # pointwise free-dim chunks: 7 x 448 (all >= 256 so f32r runs 1 cyc/col)
    PWC = 448
    PW_CHUNKS = [(i * PWC, PWC) for i in range(7)]

    with tile.TileContext(nc) as tc:
        with (
            tc.tile_pool(name='perm', bufs=1) as perm,
            tc.tile_pool(name='kpool', bufs=6) as kpool,
            tc.tile_pool(name='k2p', bufs=3) as k2p,
            tc.tile_pool(name='dram', bufs=1, space='DRAM') as dpool,
            tc.tile_pool(name='tp', bufs=2, space='PSUM') as tpp,
        ):
            # ------------------------------------------------ constants
            ident = perm.tile([P, P], f32, name='identsb')
            nc.sync.dma_start(ident[:], ident_in[:])
            # warmup collective: absorbs the CC cold-start latency
            # while phase A runs
            wu = perm.tile([1, 2], f32, name='wu')
            nc.vector.memset(wu[:], 0.0)
            ag0_in = dpool.tile([2], f32, name='ag0_in')
            ag0_out = dpool.tile([NCORES * 2], f32, name='ag0_out')
            nc.sync.dma_start(ag0_in[None, :], wu[:])
            nc.gpsimd.collective_compute(
                'AllGather', AL.bypass, replica_groups=rg,
                ins=[ag0_in[:].opt()], outs=[ag0_out[:].opt()])
            qdw = perm.tile([P, G, 9], f32, name='qdwsb')
            nc.sync.dma_start(qdw[:], qdw_in.rearrange('g c t -> c g t'))

            def load_gp(t_in, nm):
                t = perm.tile([P, G], f32, name=nm)
                nc.sync.dma_start(t[:], t_in.rearrange('g c -> c g'))
                return t
            wsum_t = load_gp(wsum_in, 'wsumsb')
            qdb_t = load_gp(qdb_in, 'qdbsb')
            qbn1w_t = load_gp(qbn1w_in, 'qbn1wsb')
            bn1b_t = load_gp(bn1b_in, 'bn1bsb')
            qbn2w_t = load_gp(qbn2w_in, 'qbn2wsb')
            bn2b_t = load_gp(bn2b_in, 'bn2bsb')
            pwsum_t = load_gp(pwsum_in, 'pwsumsb')
            pwT = perm.tile([P, G, 256], f32, name='pwTsb')
            nc.sync.dma_start(pwT[:], pwT_in[:].rearrange('g c m -> c g m'))

            # diag weight matrices (filled after AG1: qdw * s_x folded in)
            diagt = perm.tile([P, G * 9 * P], f32r, name='diagt')

            # ------------------------------------------------ stat tiles
            # per-(c,b): [min(BL) | max(BL) | sum(BL)]
            xstat = [perm.tile([P, 2 * BL], f32, name=f'xstat{g}')
                     for g in range(G)]
            h1stat = [perm.tile([P, 3 * BL], f32, name=f'h1stat{g}')
                      for g in range(G)]
            h3stat = [perm.tile([P, 3 * BL], f32, name=f'h3stat{g}')
                      for g in range(G)]
            hsum8 = [perm.tile([P, BL, 8], f32, name=f'hsum8_{g}')
                     for g in range(G)]
            Ag = [perm.tile([P, 3, NCORES, 2, 2], f32, name=f'Ag{g}')
                  for g in range(G)]
            cstat = [perm.tile([P, 2, NCORES * 2], f32, name=f'cstat{g}')
                     for g in range(G)]

            def sc1(nm):
                return perm.tile([1, 1], f32, name=nm)

            def bc1(nm):
                return perm.tile([P, 1], f32, name=nm)

            # scalar math helper: from sum-of-mins/sum-of-maxes -> quant params
            def quant_params(mnsum, mxsum, tag):
                mn = sc1(f'mn_{tag}')
                mx = sc1(f'mx_{tag}')
                nc.vector.tensor_scalar(mn[:], mnsum[:], 1.0 / B_FULL, None, op0=AL.mult)
                nc.vector.tensor_scalar(mx[:], mxsum[:], 1.0 / B_FULL, None, op0=AL.mult)
                d = sc1(f'd_{tag}')
                nc.vector.tensor_sub(d[:], mx[:], mn[:])
                s = sc1(f's_{tag}')
                nc.vector.tensor_scalar(s[:], d[:], 1.0 / QMAX, 1e-8,
                                        op0=AL.mult, op1=AL.max)
                inv_s = sc1(f'invs_{tag}')
                nc.vector.reciprocal(inv_s[:], s[:])
                negmn = sc1(f'negmn_{tag}')
                nc.vector.tensor_scalar(negmn[:], mn[:], -1.0, None, op0=AL.mult)
                bias = sc1(f'bias_{tag}')
                nc.vector.tensor_mul(bias[:], negmn[:], inv_s[:])
                return {'mn': mn, 'mx': mx, 's': s, 'inv_s': inv_s,
                        'negmn': negmn, 'bias': bias}

            def bcast(src, nm):
                t = bc1(nm)
                nc.gpsimd.partition_broadcast(t[:], src[:])
                return t

            # quantize chain on a small [P, F] stat tile (value form k, fp32)
            def qchain_small(ap, inv_s_bc, bias_bc):
                nc.scalar.activation(ap, ap, AF.Relu, bias=bias_bc[:, 0:1],
                                     scale=inv_s_bc[:, 0:1])
                nc.vector.tensor_scalar(ap, ap, QMAX, MAGIC, op0=AL.min, op1=AL.add)
                nc.vector.tensor_scalar(ap, ap, MAGIC, None, op0=AL.subtract)

            def minmax_tree(nm, src_ap, mn_ap, mx_ap):
                nc.vector.tensor_reduce(mn_ap, src_ap, axis=AX.X, op=AL.min)
                nc.vector.tensor_reduce(mx_ap, src_ap, axis=AX.X, op=AL.max)

            # =================================================================
            # Stage A: load x (contiguous) + x stats
            # =================================================================
            with tc.tile_pool(name='img', bufs=11) as img:
                raw = {}
                for b in range(BL):
                    for g in range(G):
                        t = img.tile([P, IMG], f32, name=f'raw{g}_{b}',
                                     tag='img')
                        raw[(g, b)] = t
                        nc.sync.dma_start(
                            t[:].rearrange('p (h w) -> p h w', h=HH),
                            x_in[b, g * P:(g + 1) * P])
                        minmax_tree(f'xs{g}_{b}', t[:],
                                    xstat[g][:, b:b + 1],
                                    xstat[g][:, BL + b:BL + b + 1])

                # --- AG1: per-sample min/max (8 floats per core) ---
                tmin = tpp.tile([BL, G * P], f32, name='tmin1', tag='tp')
                tmax = tpp.tile([BL, G * P], f32, name='tmax1', tag='tp')
                for g in range(G):
                    nc.tensor.transpose(tmin[:, g * P:(g + 1) * P],
                                        xstat[g][:, 0:BL], ident[:])
                    nc.tensor.transpose(tmax[:, g * P:(g + 1) * P],
                                        xstat[g][:, BL:2 * BL], ident[:])
                ab1 = perm.tile([BL, 2], f32, name='ab1')
                nc.vector.tensor_reduce(ab1[:, 0:1], tmin[:], axis=AX.X, op=AL.min)
                nc.vector.tensor_reduce(ab1[:, 1:2], tmax[:], axis=AX.X, op=AL.max)

                ag1_in = dpool.tile([BL * 2], f32, name='ag1_in')
                ag1_out = dpool.tile([NCORES * BL * 2], f32, name='ag1_out')
                nc.sync.dma_start(ag1_in.rearrange('(b s) -> b s', s=2), ab1[:])
                nc.gpsimd.collective_compute(
                    'AllGather', AL.bypass, replica_groups=rg,
                    ins=[ag1_in[:].opt()], outs=[ag1_out[:].opt()])
                agb1 = perm.tile([1, NCORES * BL * 2], f32, name='agb1')
                nc.sync.dma_start(agb1[:], ag1_out[None, :])
                v1 = agb1.rearrange('p (cb s) -> p s cb', s=2)
                mnsum_x = sc1('mnsum_x')
                mxsum_x = sc1('mxsum_x')
                nc.vector.tensor_reduce(mnsum_x[:], v1[:, 0, :], axis=AX.X, op=AL.add)
                nc.vector.tensor_reduce(mxsum_x[:], v1[:, 1, :], axis=AX.X, op=AL.add)
                qx = quant_params(mnsum_x, mxsum_x, 'x')
                invsx_bc = bcast(qx['inv_s'], 'invsx_bc')
                biasx_bc = bcast(qx['bias'], 'biasx_bc')
                sx_bc = bcast(qx['s'], 'sx_bc')
                # centered k' = k - 128 halves |k| so f32r operand rounding
                # of the products shrinks ~5x.  border k' = -mn/s - 128;
                # wsum correction uses mn' = mn + 128*s.
                nmos128 = sc1('nmos128')
                nc.vector.tensor_scalar(nmos128[:], qx['bias'][:], -128.0,
                                        None, op0=AL.add)
                nmos_bc = bcast(nmos128, 'nmos_bc')
                mnp = sc1('mnp')
                nc.vector.tensor_scalar(mnp[:], qx['s'][:], 128.0,
                                        qx['mn'][:, 0:1],
                                        op0=AL.mult, op1=AL.add)
                mnx_bc = bcast(mnp, 'mnx_bc')

                # diag weights: ident * (qdw[c,g,t] * s_x)
                for g in range(G):
                    for t in range(9):
                        i = g * 9 + t
                        nc.vector.tensor_scalar(
                            diagt[:, i * P:(i + 1) * P], ident[:],
                            qdw[:, g, t:t + 1], sx_bc[:, 0:1],
                            op0=AL.mult, op1=AL.mult)

                # const1 = qdb + mn_x * wsum  (per channel)
                const1 = perm.tile([P, G], f32, name='const1')
                for g in range(G):
                    nc.vector.scalar_tensor_tensor(
                        const1[:, g:g + 1], wsum_t[:, g:g + 1], mnx_bc[:, 0:1],
                        qdb_t[:, g:g + 1], op0=AL.mult, op1=AL.add)

                # constant border strip: value -mn/s everywhere
                bord = perm.tile([P, PADW], f32r, name='bord')
                nc.vector.tensor_scalar(bord[:], ident[:, 0:PADW], 0.0,
                                        nmos_bc[:, 0:1],
                                        op0=AL.mult, op1=AL.add)

                # =========================================================
                # Stage B+C per tile: quantize into padded tile, conv, evict
                # =========================================================
                h1 = {}
                with tc.tile_pool(name='cv', bufs=6, space='PSUM') as cvp:
                    for b in range(BL):
                        for g in range(G):
                            rt = raw[(g, b)]
                            # B: k = round(clip((x-mn)/s)) via saturating
                            # u8 convert (RNE), then center to k-128 (f32r)
                            ku = kpool.tile([P, IMG], u8, name=f'kx{g}_{b}',
                                            tag='k8')
                            nc.scalar.activation(ku[:], rt[:], AF.Relu,
                                                 bias=biasx_bc[:, 0:1],
                                                 scale=invsx_bc[:, 0:1])
                            xp = img.tile([P, PADI], f32r, name=f'xp{g}_{b}',
                                          tag='img')
                            v = xp.rearrange('p (h w) -> p h w', h=PADW)
                            nc.vector.tensor_scalar(
                                v[:, 1:57, 1:57],
                                ku[:].rearrange('p (h w) -> p h w', h=HH),
                                128.0, None, op0=AL.subtract)
                            # borders := -mn/s (cancels wsum correction)
                            for bap, bw in ((v[:, 0, :], PADW),
                                            (v[:, 57, :], PADW),
                                            (v[:, 1:57, 0], HH),
                                            (v[:, 1:57, 57], HH)):
                                nc.vector.tensor_scalar(bap, bord[:, 0:bw],
                                                        1.0, None, op0=AL.mult)
                            # C: depthwise conv via diag f32r matmuls
                            src = v
                            h1t = img.tile([P, IMG], f32, name=f'h1_{g}_{b}',
                                           tag='img')
                            h1[(g, b)] = h1t
                            for half in range(2):
                                pst = [cvp.tile([P, 392], f32,
                                                name=f'cv{g}{b}{half}{rb}',
                                                tag='cv')
                                       for rb in range(4)]
                                for t in range(9):
                                    di, dj = t // 3, t % 3
                                    lhs = diagt[:, (g * 9 + t) * P:(g * 9 + t + 1) * P]
                                    for rb in range(4):
                                        r0 = half * 28 + rb * 7
                                        rhs = src[:, r0 + di:r0 + di + 7,
                                                  dj:dj + 56]
                                        nc.tensor.matmul(
                                            pst[rb][:], lhs, rhs,
                                            start=(t == 0), stop=(t == 8))
                                for rb in range(4):
                                    r0 = half * 28 + rb * 7
                                    j = half * 4 + rb
                                    nc.scalar.activation(
                                        h1t[:, r0 * HH:(r0 + 7) * HH],
                                        pst[rb][:], AF.Identity,
                                        bias=const1[:, g:g + 1], scale=1.0,
                                        accum_out=hsum8[g][:, b, j:j + 1])
                            minmax_tree(f'h1s{g}_{b}', h1t[:],
                                        h1stat[g][:, b:b + 1],
                                        h1stat[g][:, BL + b:BL + b + 1])
                            nc.vector.tensor_reduce(
                                h1stat[g][:, 2 * BL + b:2 * BL + b + 1],
                                hsum8[g][:, b], axis=AX.X, op=AL.add)

                # =========================================================
                # AG2: per-(channel,sample) h1 min/max/sum
                # =========================================================
                ag2_in = dpool.tile([G * P * 3 * BL], f32, name='ag2_in')
                ag2_out = dpool.tile([NCORES * G * P * 3 * BL], f32, name='ag2_out')
                v2i = ag2_in.rearrange('(g c f) -> g c f', g=G, c=P)
                for g in range(G):
                    nc.sync.dma_start(v2i[g], h1stat[g][:])
                nc.gpsimd.collective_compute(
                    'AllGather', AL.bypass, replica_groups=rg,
                    ins=[ag2_in[:].opt()], outs=[ag2_out[:].opt()])
                v2o = ag2_out.rearrange(
                    '(core g c s q b) -> g c s core q b',
                    core=NCORES, g=G, c=P, s=3, q=2)
                for g in range(G):
                    for s in range(3):
                        nc.sync.dma_start(Ag[g][:, s], v2o[g][:, s])

                # chunk stats (pair of batches within a core)
                for g in range(G):
                    nc.vector.tensor_reduce(
                        cstat[g][:, 0, :], Ag[g][:, 0], axis=AX.X, op=AL.min)
                    nc.vector.tensor_reduce(
                        cstat[g][:, 1, :], Ag[g][:, 1], axis=AX.X, op=AL.max)

                # per-sample min/max over all 256 channels -> qm() params
                def sample_params(stattiles, tag):
                    tmn = tpp.tile([B_FULL, G * P], f32, name=f'tmn_{tag}', tag='tp')
                    tmx = tpp.tile([B_FULL, G * P], f32, name=f'tmx_{tag}', tag='tp')
                    for g in range(G):
                        flat = stattiles[g].rearrange('p s core q b -> p (s core q b)')
                        nc.tensor.transpose(tmn[:, g * P:(g + 1) * P],
                                            flat[:, 0:B_FULL], ident[:])
                        nc.tensor.transpose(tmx[:, g * P:(g + 1) * P],
                                            flat[:, B_FULL:2 * B_FULL], ident[:])
                    pm = perm.tile([B_FULL, 2], f32, name=f'pm_{tag}')
                    nc.vector.tensor_reduce(pm[:, 0:1], tmn[:], axis=AX.X, op=AL.min)
                    nc.vector.tensor_reduce(pm[:, 1:2], tmx[:], axis=AX.X, op=AL.max)
                    ta = tpp.tile([1, B_FULL], f32, name=f'ta_{tag}', tag='tp')
                    tb = tpp.tile([1, B_FULL], f32, name=f'tb_{tag}', tag='tp')
                    nc.tensor.transpose(ta[:], pm[:, 0:1], ident[0:B_FULL, 0:B_FULL])
                    nc.tensor.transpose(tb[:], pm[:, 1:2], ident[0:B_FULL, 0:B_FULL])
                    mnsum = sc1(f'mnsum_{tag}')
                    mxsum = sc1(f'mxsum_{tag}')
                    nc.vector.tensor_reduce(mnsum[:], ta[:], axis=AX.X, op=AL.add)
                    nc.vector.tensor_reduce(mxsum[:], tb[:], axis=AX.X, op=AL.add)
                    return quant_params(mnsum, mxsum, tag)

                q1 = sample_params(Ag, 'h1')
                invs1_bc = bcast(q1['inv_s'], 'invs1_bc')
                bias1_bc = bcast(q1['bias'], 'bias1_bc')
                s1_bc = bcast(q1['s'], 's1_bc')
                mn1_bc = bcast(q1['mn'], 'mn1_bc')

                # RangeBN scale from chunk stats
                def rangebn_scale(cstat_g, invs_bc, bias_bc, s_bc, mn_bc, tag):
                    scpk = perm.tile([P, G], f32, name=f'scpk_{tag}')
                    for g in range(G):
                        c = cstat_g[g].rearrange('p s f -> p (s f)')
                        qchain_small(c[:, :], invs_bc, bias_bc)
                        # now c holds integer k; mean over 16 chunks, value form
                        mm = perm.tile([P, 2], f32, name=f'mm_{tag}{g}')
                        nc.vector.tensor_reduce(
                            mm[:], cstat_g[g][:], axis=AX.X, op=AL.add)
                        # mm = (sum k)/16 * s + mn
                        nc.vector.tensor_scalar(mm[:], mm[:], 1.0 / NCHUNKS,
                                                s_bc[:, 0:1],
                                                op0=AL.mult, op1=AL.mult)
                        nc.vector.tensor_scalar(mm[:], mm[:], mn_bc[:, 0:1],
                                                None, op0=AL.add)
                        d = perm.tile([P, 1], f32, name=f'dmm_{tag}{g}')
                        nc.vector.tensor_sub(d[:], mm[:, 1:2], mm[:, 0:1])
                        nc.vector.tensor_scalar(d[:], d[:], SCALE_FIX, EPS,
                                                op0=AL.mult, op1=AL.add)
                        nc.vector.reciprocal(scpk[:, g:g + 1], d[:])
                    # quantize scale over all 256 channels
                    tq = tpp.tile([1, G * P], f32, name=f'tq_{tag}', tag='tp')
                    for g in range(G):
                        nc.tensor.transpose(tq[:, g * P:(g + 1) * P],
                                            scpk[:, g:g + 1], ident[:])
                    smn = sc1(f'smn_{tag}')
                    smx = sc1(f'smx_{tag}')
                    nc.vector.tensor_reduce(smn[:], tq[:], axis=AX.X, op=AL.min)
                    nc.vector.tensor_reduce(smx[:], tq[:], axis=AX.X, op=AL.max)
                    dd = sc1(f'sd_{tag}')
                    nc.vector.tensor_sub(dd[:], smx[:], smn[:])
                    ss = sc1(f'ss_{tag}')
                    nc.vector.tensor_scalar(ss[:], dd[:], 1.0 / QMAX, 1e-8,
                                            op0=AL.mult, op1=AL.max)
                    invss = sc1(f'invss_{tag}')
                    nc.vector.reciprocal(invss[:], ss[:])
                    negsmn = sc1(f'negsmn_{tag}')
                    nc.vector.tensor_scalar(negsmn[:], smn[:], -1.0, None, op0=AL.mult)
                    bss = sc1(f'bss_{tag}')
                    nc.vector.tensor_mul(bss[:], negsmn[:], invss[:])
                    invss_bc = bcast(invss, f'invss_bc_{tag}')
                    bss_bc = bcast(bss, f'bss_bc_{tag}')
                    ss_bc = bcast(ss, f'ss_bc_{tag}')
                    smn_bc = bcast(smn, f'smn_bc_{tag}')
                    qchain_small(scpk[:, :], invss_bc, bss_bc)
                    nc.vector.tensor_scalar(scpk[:], scpk[:], ss_bc[:, 0:1],
                                            None, op0=AL.mult)
                    nc.vector.tensor_scalar(scpk[:], scpk[:], smn_bc[:, 0:1],
                                            None, op0=AL.add)
                    return scpk

                qscale1 = rangebn_scale(cstat, invs1_bc, bias1_bc, s1_bc,
                                        mn1_bc, 'bn1')
                A1 = perm.tile([P, G], f32, name='A1')
                nc.vector.tensor_mul(A1[:], qscale1[:], qbn1w_t[:])
                cA1 = perm.tile([P, G], f32, name='cA1')
                nc.vector.tensor_scalar(cA1[:], A1[:], s1_bc[:, 0:1], None,
                                        op0=AL.mult)

                # mean1 = (sum over cores+batches of h1 sums) / N_TOT
                mean1 = perm.tile([P, G], f32, name='mean1')
                for g in range(G):
                    fsum = Ag[g].rearrange('p s core q b -> p s (core q b)')
                    nc.vector.tensor_reduce(mean1[:, g:g + 1],
                                            fsum[:, 2, :], axis=AX.X, op=AL.add)
                nc.vector.tensor_scalar(mean1[:], mean1[:], 1.0 / N_TOT, None,
                                        op0=AL.mult)
                cB1 = perm.tile([P, G], f32, name='cB1')
                nc.vector.tensor_scalar(cB1[:], mean1[:], -1.0,
                                        mn1_bc[:, 0:1], op0=AL.mult, op1=AL.add)
                # cB1 currently = (mn1 - mean1); multiply by A1, add bn1b
                nc.vector.tensor_mul(cB1[:], cB1[:], A1[:])
                nc.vector.tensor_add(cB1[:], cB1[:], bn1b_t[:])

                # analytic qm(h2) bounds from Ag (monotone: cA1 >= 0)
                for g in range(G):
                    flat = Ag[g].rearrange('p s core q b -> p (s core q b)')
                    ext = flat[:, 0:2 * B_FULL]
                    qchain_small(ext, invs1_bc, bias1_bc)
                    nc.scalar.activation(ext, ext,
                                         AF.Relu, bias=cB1[:, g:g + 1],
                                         scale=cA1[:, g:g + 1])
                q2 = sample_params(Ag, 'h2')
                invs2_bc = bcast(q2['inv_s'], 'invs2_bc')
                bias2_bc = bcast(q2['bias'], 'bias2_bc')
                s2_bc = bcast(q2['s'], 's2_bc')
                mn2_bc = bcast(q2['mn'], 'mn2_bc')

                # E-stage fused affine: k2 = round(clip(aE*k1 + bE, 0, 255))
                aE = perm.tile([P, G], f32, name='aE')
                bE = perm.tile([P, G], f32, name='bE')
                nc.vector.tensor_scalar(aE[:], cA1[:], invs2_bc[:, 0:1], None,
                                        op0=AL.mult)
                nc.vector.tensor_scalar(bE[:], cB1[:], mn2_bc[:, 0:1],
                                        invs2_bc[:, 0:1],
                                        op0=AL.subtract, op1=AL.mult)

                # scaled pointwise weights (bf16: k2 integers exact, weight
                # rounding in the 256-way contraction is harmless) + const3
                pwTs = perm.tile([P, G, 256], bf16, name='pwTs')
                nc.vector.tensor_scalar(pwTs[:], pwT[:], s2_bc[:, 0:1], None,
                                        op0=AL.mult)
                const3 = perm.tile([P, G], f32, name='const3')
                nc.vector.tensor_scalar(const3[:], pwsum_t[:], mn2_bc[:, 0:1],
                                        None, op0=AL.mult)

                # =========================================================
                # Stages D+E+F per batch: h1 -> k1(u8) -> k2(u8) -> bf16
                # -> h3 (SBUF); Sum(h3) accumulated during eviction
                # =========================================================
                h3 = {}
                with tc.tile_pool(name='pw', bufs=6, space='PSUM') as pwp:
                    for b in range(BL):
                        k2b = {}
                        for g in range(G):
                            # D: k1 = round(clip((h1-mn1)/s1)) via the
                            # saturating RNE u8 convert -- one ACT op
                            kt = kpool.tile([P, IMG], u8, name=f'k1_{g}_{b}',
                                            tag='k8')
                            nc.scalar.activation(kt[:], h1[(g, b)][:], AF.Relu,
                                                 bias=bias1_bc[:, 0:1],
                                                 scale=invs1_bc[:, 0:1])
                            # E: k2 = round(clip(aE*k1 + bE)), in place u8->u8
                            nc.scalar.activation(kt[:], kt[:], AF.Relu,
                                                 bias=bE[:, g:g + 1],
                                                 scale=aE[:, g:g + 1])
                            # convert to bf16 for the pointwise matmuls
                            k2t = k2p.tile([P, IMG], bf16, name=f'k2_{g}_{b}',
                                           tag='k2')
                            nc.vector.tensor_scalar(k2t[:], kt[:], 1.0, None,
                                                    op0=AL.mult)
                            k2b[g] = k2t
                        # F: pointwise conv for this batch (bf16)
                        for cg in range(G):
                            h3t = img.tile([P, IMG], f32, name=f'h3_{cg}_{b}',
                                           tag='img')
                            h3[(cg, b)] = h3t
                            for blk in (PW_CHUNKS[0:4], PW_CHUNKS[4:7]):
                                pst = {}
                                for (c0, nn) in blk:
                                    pst[c0] = pwp.tile([P, PWC], f32,
                                                       name=f'pw{cg}{b}{c0}',
                                                       tag='pw')
                                for kg in range(G):
                                    lhs = pwTs[:, kg, cg * P:(cg + 1) * P]
                                    for (c0, nn) in blk:
                                        nc.tensor.matmul(
                                            pst[c0][:, 0:nn], lhs,
                                            k2b[kg][:, c0:c0 + nn],
                                            start=(kg == 0), stop=(kg == 1))
                                for ji, (c0, nn) in enumerate(blk):
                                    j = (0 if c0 < 4 * PWC else 4) + ji
                                    nc.scalar.activation(
                                        h3t[:, c0:c0 + nn], pst[c0][:, 0:nn],
                                        AF.Identity, bias=const3[:, cg:cg + 1],
                                        scale=1.0,
                                        accum_out=hsum8[cg][:, b, j:j + 1])
                            minmax_tree(f'h3s{cg}_{b}', h3t[:],
                                        h3stat[cg][:, b:b + 1],
                                        h3stat[cg][:, BL + b:BL + b + 1])
                            nc.vector.tensor_reduce(
                                h3stat[cg][:, 2 * BL + b:2 * BL + b + 1],
                                hsum8[cg][:, b, 0:7], axis=AX.X, op=AL.add)

                # =========================================================
                # AG5 + RangeBN2 stats
                # =========================================================
                ag5_in = dpool.tile([G * P * 3 * BL], f32, name='ag5_in')
                ag5_out = dpool.tile([NCORES * G * P * 3 * BL], f32,
                                     name='ag5_out')
                v5i = ag5_in.rearrange('(g c f) -> g c f', g=G, c=P)
                for g in range(G):
                    nc.sync.dma_start(v5i[g], h3stat[g][:])
                nc.gpsimd.collective_compute(
                    'AllGather', AL.bypass, replica_groups=rg,
                    ins=[ag5_in[:].opt()], outs=[ag5_out[:].opt()])
                v5o = ag5_out.rearrange(
                    '(core g c s q b) -> g c s core q b',
                    core=NCORES, g=G, c=P, s=3, q=2)
                for g in range(G):
                    for s in range(3):
                        nc.sync.dma_start(Ag[g][:, s], v5o[g][:, s])
                for g in range(G):
                    nc.vector.tensor_reduce(
                        cstat[g][:, 0, :], Ag[g][:, 0], axis=AX.X, op=AL.min)
                    nc.vector.tensor_reduce(
                        cstat[g][:, 1, :], Ag[g][:, 1], axis=AX.X, op=AL.max)
                q3 = sample_params(Ag, 'h3')
                invs3_bc = bcast(q3['inv_s'], 'invs3_bc')
                bias3_bc = bcast(q3['bias'], 'bias3_bc')
                s3_bc = bcast(q3['s'], 's3_bc')
                mn3_bc = bcast(q3['mn'], 'mn3_bc')
                qscale3 = rangebn_scale(cstat, invs3_bc, bias3_bc, s3_bc,
                                        mn3_bc, 'bn2')
                A3 = perm.tile([P, G], f32, name='A3')
                nc.vector.tensor_mul(A3[:], qscale3[:], qbn2w_t[:])
                cA3 = perm.tile([P, G], f32, name='cA3')
                nc.vector.tensor_scalar(cA3[:], A3[:], s3_bc[:, 0:1], None,
                                        op0=AL.mult)
                mean3 = perm.tile([P, G], f32, name='mean3')
                for g in range(G):
                    fsum = Ag[g].rearrange('p s core q b -> p s (core q b)')
                    nc.vector.tensor_reduce(mean3[:, g:g + 1],
                                            fsum[:, 2, :], axis=AX.X, op=AL.add)
                nc.vector.tensor_scalar(mean3[:], mean3[:], 1.0 / N_TOT, None,
                                        op0=AL.mult)
                cB3 = perm.tile([P, G], f32, name='cB3')
                nc.vector.tensor_scalar(cB3[:], mean3[:], -1.0,
                                        mn3_bc[:, 0:1], op0=AL.mult, op1=AL.add)
                nc.vector.tensor_mul(cB3[:], cB3[:], A3[:])
                nc.vector.tensor_add(cB3[:], cB3[:], bn2b_t[:])

                # =========================================================
                # Stages G+H per tile: h3 -> k3 (in place) -> out -> DMA
                # =========================================================
                for b in range(BL):
                    for g in range(G):
                        # G: k3 = round(clip((h3-mn3)/s3)) via u8 convert
                        kt3 = kpool.tile([P, IMG], u8, name=f'k3_{g}_{b}',
                                         tag='k8')
                        nc.scalar.activation(kt3[:], h3[(g, b)][:], AF.Relu,
                                             bias=bias3_bc[:, 0:1],
                                             scale=invs3_bc[:, 0:1])
                        # H on DVE: out = relu(cA3*k3 + cB3)
                        ot = img.tile([P, IMG], f32, name=f'out_{g}_{b}',
                                      tag='img')
                        nc.vector.tensor_scalar(ot[:], kt3[:],
                                                cA3[:, g:g + 1],
                                                cB3[:, g:g + 1],
                                                op0=AL.mult, op1=AL.add)
                        nc.vector.tensor_scalar(ot[:], ot[:], 0.0, None,
                                                op0=AL.max)
                        nc.sync.dma_start(
                            out_d[b, g * P:(g + 1) * P].rearrange(
                                'c h w -> c (h w)'), ot[:])

    nc.compile()
    return nc


def _host_consts(dw_w, dw_b, bn1_w, bn1_b, pw_w, bn2_w, bn2_b):
    qdw = _host_quant(dw_w).reshape(256, 9)
    qdb = _host_quant(dw_b)
    qpw = _host_quant(pw_w).reshape(256, 256)
    qbn1w = _host_quant(bn1_w)
    qbn2w = _host_quant(bn2_w)
    wsum = qdw.sum(axis=1, dtype=np.float32)
    pwsum = qpw.sum(axis=1, dtype=np.float32)
    # lhsT layout: pwT[kg, cin, (coutg*128 + cout)] = qpw[cout_full, kg*128+cin]
    pwT = np.ascontiguousarray(
        qpw.T.reshape(G, P, 256)).astype(np.float32)
    consts = {
        'ident': np.eye(P, dtype=np.float32),
        'qdw': np.ascontiguousarray(qdw.reshape(G, P, 9)),
        'wsum': wsum.reshape(G, P).copy(),
        'qdb': qdb.reshape(G, P).copy(),
        'qbn1w': qbn1w.reshape(G, P).copy(),
        'bn1b': np.asarray(bn1_b, np.float32).reshape(G, P).copy(),
        'qbn2w': qbn2w.reshape(G, P).copy(),
        'bn2b': np.asarray(bn2_b, np.float32).reshape(G, P).copy(),
        'pwsum': pwsum.reshape(G, P).copy(),
        'pwT': pwT,
    }
    return consts


def make_in_maps(x, dw_w, dw_b, bn1_w, bn1_b, pw_w, bn2_w, bn2_b):
    x = np.asarray(x, np.float32)
    consts = _host_consts(dw_w, dw_b, bn1_w, bn1_b, pw_w, bn2_w, bn2_b)
    in_maps = []
    for c in range(NCORES):
        m = dict(consts)
        m['x'] = np.ascontiguousarray(x[c * BL:(c + 1) * BL])
        in_maps.append(m)
    return in_maps


def get_program(limit=7):
    if limit not in _PROGRAM_CACHE:
        _PROGRAM_CACHE[limit] = build_program(limit)
    return _PROGRAM_CACHE[limit]


def kernel(**inputs):
    from concourse.bass_utils import run_bass_kernel_spmd
    nc = get_program()
    in_maps = make_in_maps(**inputs)
    res = run_bass_kernel_spmd(nc, in_maps, core_ids=list(range(NCORES)))
    out = np.concatenate([res.results[i]['out'] for i in range(NCORES)],
                         axis=0)
    return out.astype(np.float32)


# revision 23
# speedup vs baseline: 1.6216x; 1.0153x over previous
"""Trainium2 Bass kernel for nn_DepthwiseSeparableFusedConv2d.

Self-contained: takes FULL inputs (x [32,256,56,56] + weights), returns FULL
output [32,256,56,56].  Data-parallel over batch across 8 NeuronCores; the
QuantMeasure / RangeBN global statistics are synchronized with small
AllGather collectives.

v2 design (per core: 4 batches, channels on partitions in 2 groups of 128):
  A:  load x contiguously, per-(channel,sample) min/max
  AG1 (per-sample min/max) -> x quant params;  diag weights scaled by s_x
  B:  quantize x -> integer k (fp32 values) written into padded tiles,
      borders = -mn/s
  C:  depthwise 3x3 conv as 9 accumulating diagonal-weight float32r matmuls;
      ACT evicts PSUM (+bias fold + channel-sum accum); DVE min/max of h1
  AG2 (per-(c,sample) h1 min/max + sums) -> qm(h1) params, RangeBN1 chunk
      stats, BN1 mean (mean over quantized h1 ~= mean h1), analytic qm(h2)
      bounds -- no extra pass, no second collective
  D:  quantize h1 -> u8 k1
  E:  k2 = round(clip(affine(k1))) as fp32 integers (BN1+requant fused)
  F:  pointwise conv k2 @ (qpw*s2) in float32r; ACT evict (+sums);
      DVE min/max of h3; h3 stays in SBUF (no DRAM spill)
  AG5 -> qm(h3) params + RangeBN2 stats + BN2 mean
  G:  requantize h3 -> k3 (in place);  H: out = relu(BN2(k3)) -> DMA out
"""

import math
import numpy as np

# ---------------------------------------------------------------- constants
P = 128
G = 2                 # channel groups (256 = 2*128)
B_FULL = 32
BL = 4                # batches per core
NCORES = 8
HH = 56
IMG = HH * HH         # 3136
PADW = 58
PADI = PADW * PADW    # 3364
MAGIC = 12582912.0    # 1.5 * 2**23  (fp32 round-to-nearest-even trick)
QMAX = 255.0
N_TOT = B_FULL * IMG  # 100352
NCHUNKS = 16
EPS = 1e-5
_N_CHUNK_EL = B_FULL * IMG // NCHUNKS
SCALE_FIX = float((0.5 * 0.35) * (1 + (math.pi * math.log(4)) ** 0.5)
                  / ((2 * math.log(_N_CHUNK_EL)) ** 0.5))

_PROGRAM_CACHE = {}


def _host_quant(w):
    w = np.asarray(w, np.float32)
    mn = w.min()
    mx = w.max()
    scale = np.maximum(((mx - mn) / np.float32(QMAX)).astype(np.float32),
                       np.float32(1e-8))
    t = np.clip((w - mn) / scale, np.float32(0.0), np.float32(QMAX)).astype(np.float32)
    return (np.round(t) * scale + mn).astype(np.float32)


def build_program(limit=7):  # limit unused in full build
    import concourse.bacc as bacc
    import concourse.mybir as mybir
    import concourse.tile as tile

    f32 = mybir.dt.float32
    f32r = mybir.dt.float32r
    bf16 = mybir.dt.bfloat16
    u8 = mybir.dt.uint8
    AL = mybir.AluOpType
    AF = mybir.ActivationFunctionType
    AX = mybir.AxisListType

    nc = bacc.Bacc('TRN2', target_bir_lowering=False, debug=False,
                   num_devices=NCORES)

    # ------------------------------------------------ external tensors
    x_in = nc.dram_tensor('x', [BL, 256, HH, HH], f32, kind='ExternalInput')
    ident_in = nc.dram_tensor('ident', [P, P], f32, kind='ExternalInput')
    qdw_in = nc.dram_tensor('qdw', [G, P, 9], f32, kind='ExternalInput')
    wsum_in = nc.dram_tensor('wsum', [G, P], f32, kind='ExternalInput')
    qdb_in = nc.dram_tensor('qdb', [G, P], f32, kind='ExternalInput')
    qbn1w_in = nc.dram_tensor('qbn1w', [G, P], f32, kind='ExternalInput')
    bn1b_in = nc.dram_tensor('bn1b', [G, P], f32, kind='ExternalInput')
    qbn2w_in = nc.dram_tensor('qbn2w', [G, P], f32, kind='ExternalInput')
    bn2b_in = nc.dram_tensor('bn2b', [G, P], f32, kind='ExternalInput')
    pwsum_in = nc.dram_tensor('pwsum', [G, P], f32, kind='ExternalInput')
    # pwT[kg, cin(128), (coutg, cout)] : lhsT layout, already transposed
    pwT_in = nc.dram_tensor('pwT', [G, P, 256], f32, kind='ExternalInput')
    out_d = nc.dram_tensor('out', [BL, 256, HH, HH], f32, kind='ExternalOutput')

    rg = [list(range(NCORES))]

    # pointwise free-dim chunks: 7 x 448 (all >= 256 so f32r runs 1 cyc/col)
    PWC = 448
    PW_CHUNKS = [(i * PWC, PWC) for i in range(7)]

    with tile.TileContext(nc) as tc:
        with (
            tc.tile_pool(name='perm', bufs=1) as perm,
            tc.tile_pool(name='kpool', bufs=6) as kpool,
            tc.tile_pool(name='k2p', bufs=3) as k2p,
            tc.tile_pool(name='dram', bufs=1, space='DRAM') as dpool,
        ):
            # ------------------------------------------------ constants
            ident = perm.tile([P, P], f32, name='identsb')
            nc.sync.dma_start(ident[:], ident_in[:])
            # warmup collective: absorbs the CC cold-start latency
            # while phase A runs
            wu = perm.tile([1, 2], f32, name='wu')
            nc.vector.memset(wu[:], 0.0)
            ag0_in = dpool.tile([2], f32, name='ag0_in')
            ag0_out = dpool.tile([NCORES * 2], f32, name='ag0_out')
            nc.sync.dma_start(ag0_in[None, :], wu[:])
            nc.gpsimd.collective_compute(
                'AllGather', AL.bypass, replica_groups=rg,
                ins=[ag0_in[:].opt()], outs=[ag0_out[:].opt()])
            qdw = perm.tile([P, G, 9], f32, name='qdwsb')
            nc.sync.dma_start(qdw[:], qdw_in.rearrange('g c t -> c g t'))

            def load_gp(t_in, nm):
                t = perm.tile([P, G], f32, name=nm)
                nc.sync.dma_start(t[:], t_in.rearrange('g c -> c g'))
                return t
            wsum_t = load_gp(wsum_in, 'wsumsb')
            qdb_t = load_gp(qdb_in, 'qdbsb')
            qbn1w_t = load_gp(qbn1w_in, 'qbn1wsb')
            bn1b_t = load_gp(bn1b_in, 'bn1bsb')
            qbn2w_t = load_gp(qbn2w_in, 'qbn2wsb')
            bn2b_t = load_gp(bn2b_in, 'bn2bsb')
            pwsum_t = load_gp(pwsum_in, 'pwsumsb')
            pwT = perm.tile([P, G, 256], f32, name='pwTsb')
            nc.sync.dma_start(pwT[:], pwT_in[:].rearrange('g c m -> c g m'))

            # diag weight matrices (filled after AG1: qdw * s_x folded in)
            diagt = perm.tile([P, G * 9 * P], f32r, name='diagt')

            # ------------------------------------------------ stat tiles
            # per-(c,b): [min(BL) | max(BL) | sum(BL)]
            xstat = [perm.tile([P, 2 * BL], f32, name=f'xstat{g}')
                     for g in range(G)]
            h1stat = [perm.tile([P, 3 * BL], f32, name=f'h1stat{g}')
                      for g in range(G)]
            h3stat = [perm.tile([P, 3 * BL], f32, name=f'h3stat{g}')
                      for g in range(G)]
            hsum8 = [perm.tile([P, BL, 8], f32, name=f'hsum8_{g}')
                     for g in range(G)]
            Ag = [perm.tile([P, 3, NCORES, 2, 2], f32, name=f'Ag{g}')
                  for g in range(G)]
            cstat = [perm.tile([P, 2, NCORES * 2], f32, name=f'cstat{g}')
                     for g in range(G)]

            def sc1(nm):
                return perm.tile([1, 1], f32, name=nm)

            def bc1(nm):
                return perm.tile([P, 1], f32, name=nm)

            # scalar math helper: from sum-of-mins/sum-of-maxes -> quant params
            def quant_params(mnsum, mxsum, tag):
                mn = sc1(f'mn_{tag}')
                mx = sc1(f'mx_{tag}')
                nc.vector.tensor_scalar(mn[:], mnsum[:], 1.0 / B_FULL, None, op0=AL.mult)
                nc.vector.tensor_scalar(mx[:], mxsum[:], 1.0 / B_FULL, None, op0=AL.mult)
                d = sc1(f'd_{tag}')
                nc.vector.tensor_sub(d[:], mx[:], mn[:])
                s = sc1(f's_{tag}')
                nc.vector.tensor_scalar(s[:], d[:], 1.0 / QMAX, 1e-8,
                                        op0=AL.mult, op1=AL.max)
                inv_s = sc1(f'invs_{tag}')
                nc.vector.reciprocal(inv_s[:], s[:])
                negmn = sc1(f'negmn_{tag}')
                nc.vector.tensor_scalar(negmn[:], mn[:], -1.0, None, op0=AL.mult)
                bias = sc1(f'bias_{tag}')
                nc.vector.tensor_mul(bias[:], negmn[:], inv_s[:])
                return {'mn': mn, 'mx': mx, 's': s, 'inv_s': inv_s,
                        'negmn': negmn, 'bias': bias}

            def bcast(src, nm):
                t = bc1(nm)
                nc.gpsimd.partition_broadcast(t[:], src[:])
                return t

            # quantize chain on a small [P, F] stat tile (value form k, fp32)
            def qchain_small(ap, inv_s_bc, bias_bc):
                nc.scalar.activation(ap, ap, AF.Relu, bias=bias_bc[:, 0:1],
                                     scale=inv_s_bc[:, 0:1])
                nc.vector.tensor_scalar(ap, ap, QMAX, MAGIC, op0=AL.min, op1=AL.add)
                nc.vector.tensor_scalar(ap, ap, MAGIC, None, op0=AL.subtract)

            def minmax_tree(nm, src_ap, mn_ap, mx_ap):
                nc.vector.tensor_reduce(mn_ap, src_ap, axis=AX.X, op=AL.min)
                nc.vector.tensor_reduce(mx_ap, src_ap, axis=AX.X, op=AL.max)

            # =================================================================
            # Stage A: load x (contiguous) + x stats
            # =================================================================
            with tc.tile_pool(name='img', bufs=11) as img:
                raw = {}
                for b in range(BL):
                    for g in range(G):
                        t = img.tile([P, IMG], f32, name=f'raw{g}_{b}',
                                     tag='img')
                        raw[(g, b)] = t
                        nc.sync.dma_start(
                            t[:].rearrange('p (h w) -> p h w', h=HH),
                            x_in[b, g * P:(g + 1) * P])
                        minmax_tree(f'xs{g}_{b}', t[:],
                                    xstat[g][:, b:b + 1],
                                    xstat[g][:, BL + b:BL + b + 1])

                # --- AG1: per-sample min/max (8 floats per core) ---
                _tp1 = tc.tile_pool(name='tp1', bufs=2, space='PSUM')
                tpp = _tp1.__enter__()
                tmin = tpp.tile([BL, G * P], f32, name='tmin1', tag='tp')
                tmax = tpp.tile([BL, G * P], f32, name='tmax1', tag='tp')
                for g in range(G):
                    nc.tensor.transpose(tmin[:, g * P:(g + 1) * P],
                                        xstat[g][:, 0:BL], ident[:])
                    nc.tensor.transpose(tmax[:, g * P:(g + 1) * P],
                                        xstat[g][:, BL:2 * BL], ident[:])
                ab1 = perm.tile([BL, 2], f32, name='ab1')
                nc.vector.tensor_reduce(ab1[:, 0:1], tmin[:], axis=AX.X, op=AL.min)
                nc.vector.tensor_reduce(ab1[:, 1:2], tmax[:], axis=AX.X, op=AL.max)

                ag1_in = dpool.tile([BL * 2], f32, name='ag1_in')
                ag1_out = dpool.tile([NCORES * BL * 2], f32, name='ag1_out')
                nc.sync.dma_start(ag1_in.rearrange('(b s) -> b s', s=2), ab1[:])
                nc.gpsimd.collective_compute(
                    'AllGather', AL.bypass, replica_groups=rg,
                    ins=[ag1_in[:].opt()], outs=[ag1_out[:].opt()])
                agb1 = perm.tile([1, NCORES * BL * 2], f32, name='agb1')
                nc.sync.dma_start(agb1[:], ag1_out[None, :])
                v1 = agb1.rearrange('p (cb s) -> p s cb', s=2)
                mnsum_x = sc1('mnsum_x')
                mxsum_x = sc1('mxsum_x')
                nc.vector.tensor_reduce(mnsum_x[:], v1[:, 0, :], axis=AX.X, op=AL.add)
                nc.vector.tensor_reduce(mxsum_x[:], v1[:, 1, :], axis=AX.X, op=AL.add)
                qx = quant_params(mnsum_x, mxsum_x, 'x')
                invsx_bc = bcast(qx['inv_s'], 'invsx_bc')
                biasx_bc = bcast(qx['bias'], 'biasx_bc')
                sx_bc = bcast(qx['s'], 'sx_bc')
                # centered k' = k - 128 halves |k| so f32r operand rounding
                # of the products shrinks ~5x.  border k' = -mn/s - 128;
                # wsum correction uses mn' = mn + 128*s.
                nmos128 = sc1('nmos128')
                nc.vector.tensor_scalar(nmos128[:], qx['bias'][:], -128.0,
                                        None, op0=AL.add)
                nmos_bc = bcast(nmos128, 'nmos_bc')
                mnp = sc1('mnp')
                nc.vector.tensor_scalar(mnp[:], qx['s'][:], 128.0,
                                        qx['mn'][:, 0:1],
                                        op0=AL.mult, op1=AL.add)
                mnx_bc = bcast(mnp, 'mnx_bc')

                # diag weights: ident * (qdw[c,g,t] * s_x)
                for g in range(G):
                    for t in range(9):
                        i = g * 9 + t
                        nc.vector.tensor_scalar(
                            diagt[:, i * P:(i + 1) * P], ident[:],
                            qdw[:, g, t:t + 1], sx_bc[:, 0:1],
                            op0=AL.mult, op1=AL.mult)

                # const1 = qdb + mn_x * wsum  (per channel)
                const1 = perm.tile([P, G], f32, name='const1')
                for g in range(G):
                    nc.vector.scalar_tensor_tensor(
                        const1[:, g:g + 1], wsum_t[:, g:g + 1], mnx_bc[:, 0:1],
                        qdb_t[:, g:g + 1], op0=AL.mult, op1=AL.add)

                # constant border strip: value -mn/s everywhere
                bord = perm.tile([P, PADW], f32r, name='bord')
                nc.vector.tensor_scalar(bord[:], ident[:, 0:PADW], 0.0,
                                        nmos_bc[:, 0:1],
                                        op0=AL.mult, op1=AL.add)

                # =========================================================
                # Stage B+C per tile: quantize into padded tile, conv, evict
                # =========================================================
                _tp1.__exit__(None, None, None)
                h1 = {}
                with tc.tile_pool(name='cv', bufs=8, space='PSUM') as cvp:
                    for b in range(BL):
                        for g in range(G):
                            rt = raw[(g, b)]
                            # B: k = round(clip((x-mn)/s)) via saturating
                            # u8 convert (RNE), then center to k-128 (f32r)
                            ku = kpool.tile([P, IMG], u8, name=f'kx{g}_{b}',
                                            tag='k8')
                            nc.scalar.activation(ku[:], rt[:], AF.Relu,
                                                 bias=biasx_bc[:, 0:1],
                                                 scale=invsx_bc[:, 0:1])
                            xp = img.tile([P, PADI], f32r, name=f'xp{g}_{b}',
                                          tag='img')
                            v = xp.rearrange('p (h w) -> p h w', h=PADW)
                            nc.vector.tensor_scalar(
                                v[:, 1:57, 1:57],
                                ku[:].rearrange('p (h w) -> p h w', h=HH),
                                128.0, None, op0=AL.subtract)
                            # borders := -mn/s (cancels wsum correction)
                            for bap, bw in ((v[:, 0, :], PADW),
                                            (v[:, 57, :], PADW),
                                            (v[:, 1:57, 0], HH),
                                            (v[:, 1:57, 57], HH)):
                                nc.vector.tensor_scalar(bap, bord[:, 0:bw],
                                                        1.0, None, op0=AL.mult)
                            # C: depthwise conv via diag f32r matmuls
                            src = v
                            h1t = img.tile([P, IMG], f32, name=f'h1_{g}_{b}',
                                           tag='img')
                            h1[(g, b)] = h1t
                            for half in range(2):
                                pst = [cvp.tile([P, 392], f32,
                                                name=f'cv{g}{b}{half}{rb}',
                                                tag='cv')
                                       for rb in range(4)]
                                for t in range(9):
                                    di, dj = t // 3, t % 3
                                    lhs = diagt[:, (g * 9 + t) * P:(g * 9 + t + 1) * P]
                                    for rb in range(4):
                                        r0 = half * 28 + rb * 7
                                        rhs = src[:, r0 + di:r0 + di + 7,
                                                  dj:dj + 56]
                                        nc.tensor.matmul(
                                            pst[rb][:], lhs, rhs,
                                            start=(t == 0), stop=(t == 8))
                                for rb in range(4):
                                    r0 = half * 28 + rb * 7
                                    j = half * 4 + rb
                                    nc.scalar.activation(
                                        h1t[:, r0 * HH:(r0 + 7) * HH],
                                        pst[rb][:], AF.Identity,
                                        bias=const1[:, g:g + 1], scale=1.0,
                                        accum_out=hsum8[g][:, b, j:j + 1])
                            minmax_tree(f'h1s{g}_{b}', h1t[:],
                                        h1stat[g][:, b:b + 1],
                                        h1stat[g][:, BL + b:BL + b + 1])
                            nc.vector.tensor_reduce(
                                h1stat[g][:, 2 * BL + b:2 * BL + b + 1],
                                hsum8[g][:, b], axis=AX.X, op=AL.add)

                # =========================================================
                # AG2: per-(channel,sample) h1 min/max/sum
                # =========================================================
                _tp2 = tc.tile_pool(name='tp2', bufs=2, space='PSUM')
                tpp = _tp2.__enter__()
                ag2_in = dpool.tile([G * P * 3 * BL], f32, name='ag2_in')
                ag2_out = dpool.tile([NCORES * G * P * 3 * BL], f32, name='ag2_out')
                v2i = ag2_in.rearrange('(g c f) -> g c f', g=G, c=P)
                for g in range(G):
                    nc.sync.dma_start(v2i[g], h1stat[g][:])
                nc.gpsimd.collective_compute(
                    'AllGather', AL.bypass, replica_groups=rg,
                    ins=[ag2_in[:].opt()], outs=[ag2_out[:].opt()])
                v2o = ag2_out.rearrange(
                    '(core g c s q b) -> g c s core q b',
                    core=NCORES, g=G, c=P, s=3, q=2)
                for g in range(G):
                    for s in range(3):
                        nc.sync.dma_start(Ag[g][:, s], v2o[g][:, s])

                # chunk stats (pair of batches within a core)
                for g in range(G):
                    nc.vector.tensor_reduce(
                        cstat[g][:, 0, :], Ag[g][:, 0], axis=AX.X, op=AL.min)
                    nc.vector.tensor_reduce(
                        cstat[g][:, 1, :], Ag[g][:, 1], axis=AX.X, op=AL.max)

                # per-sample min/max over all 256 channels -> qm() params
                def sample_params(stattiles, tag):
                    tmn = tpp.tile([B_FULL, G * P], f32, name=f'tmn_{tag}', tag='tp')
                    tmx = tpp.tile([B_FULL, G * P], f32, name=f'tmx_{tag}', tag='tp')
                    for g in range(G):
                        flat = stattiles[g].rearrange('p s core q b -> p (s core q b)')
                        nc.tensor.transpose(tmn[:, g * P:(g + 1) * P],
                                            flat[:, 0:B_FULL], ident[:])
                        nc.tensor.transpose(tmx[:, g * P:(g + 1) * P],
                                            flat[:, B_FULL:2 * B_FULL], ident[:])
                    pm = perm.tile([B_FULL, 2], f32, name=f'pm_{tag}')
                    nc.vector.tensor_reduce(pm[:, 0:1], tmn[:], axis=AX.X, op=AL.min)
                    nc.vector.tensor_reduce(pm[:, 1:2], tmx[:], axis=AX.X, op=AL.max)
                    ta = tpp.tile([1, B_FULL], f32, name=f'ta_{tag}', tag='tp')
                    tb = tpp.tile([1, B_FULL], f32, name=f'tb_{tag}', tag='tp')
                    nc.tensor.transpose(ta[:], pm[:, 0:1], ident[0:B_FULL, 0:B_FULL])
                    nc.tensor.transpose(tb[:], pm[:, 1:2], ident[0:B_FULL, 0:B_FULL])
                    mnsum = sc1(f'mnsum_{tag}')
                    mxsum = sc1(f'mxsum_{tag}')
                    nc.vector.tensor_reduce(mnsum[:], ta[:], axis=AX.X, op=AL.add)
                    nc.vector.tensor_reduce(mxsum[:], tb[:], axis=AX.X, op=AL.add)
                    return quant_params(mnsum, mxsum, tag)

                q1 = sample_params(Ag, 'h1')
                invs1_bc = bcast(q1['inv_s'], 'invs1_bc')
                bias1_bc = bcast(q1['bias'], 'bias1_bc')
                s1_bc = bcast(q1['s'], 's1_bc')
                mn1_bc = bcast(q1['mn'], 'mn1_bc')

                # RangeBN scale from chunk stats
                def rangebn_scale(cstat_g, invs_bc, bias_bc, s_bc, mn_bc, tag):
                    scpk = perm.tile([P, G], f32, name=f'scpk_{tag}')
                    for g in range(G):
                        c = cstat_g[g].rearrange('p s f -> p (s f)')
                        qchain_small(c[:, :], invs_bc, bias_bc)
                        # now c holds integer k; mean over 16 chunks, value form
                        mm = perm.tile([P, 2], f32, name=f'mm_{tag}{g}')
                        nc.vector.tensor_reduce(
                            mm[:], cstat_g[g][:], axis=AX.X, op=AL.add)
                        # mm = (sum k)/16 * s + mn
                        nc.vector.tensor_scalar(mm[:], mm[:], 1.0 / NCHUNKS,
                                                s_bc[:, 0:1],
                                                op0=AL.mult, op1=AL.mult)
                        nc.vector.tensor_scalar(mm[:], mm[:], mn_bc[:, 0:1],
                                                None, op0=AL.add)
                        d = perm.tile([P, 1], f32, name=f'dmm_{tag}{g}')
                        nc.vector.tensor_sub(d[:], mm[:, 1:2], mm[:, 0:1])
                        nc.vector.tensor_scalar(d[:], d[:], SCALE_FIX, EPS,
                                                op0=AL.mult, op1=AL.add)
                        nc.vector.reciprocal(scpk[:, g:g + 1], d[:])
                    # quantize scale over all 256 channels
                    tq = tpp.tile([1, G * P], f32, name=f'tq_{tag}', tag='tp')
                    for g in range(G):
                        nc.tensor.transpose(tq[:, g * P:(g + 1) * P],
                                            scpk[:, g:g + 1], ident[:])
                    smn = sc1(f'smn_{tag}')
                    smx = sc1(f'smx_{tag}')
                    nc.vector.tensor_reduce(smn[:], tq[:], axis=AX.X, op=AL.min)
                    nc.vector.tensor_reduce(smx[:], tq[:], axis=AX.X, op=AL.max)
                    dd = sc1(f'sd_{tag}')
                    nc.vector.tensor_sub(dd[:], smx[:], smn[:])
                    ss = sc1(f'ss_{tag}')
                    nc.vector.tensor_scalar(ss[:], dd[:], 1.0 / QMAX, 1e-8,
                                            op0=AL.mult, op1=AL.max)
                    invss = sc1(f'invss_{tag}')
                    nc.vector.reciprocal(invss[:], ss[:])
                    negsmn = sc1(f'negsmn_{tag}')
                    nc.vector.tensor_scalar(negsmn[:], smn[:], -1.0, None, op0=AL.mult)
                    bss = sc1(f'bss_{tag}')
                    nc.vector.tensor_mul(bss[:], negsmn[:], invss[:])
                    invss_bc = bcast(invss, f'invss_bc_{tag}')
                    bss_bc = bcast(bss, f'bss_bc_{tag}')
                    ss_bc = bcast(ss, f'ss_bc_{tag}')
                    smn_bc = bcast(smn, f'smn_bc_{tag}')
                    qchain_small(scpk[:, :], invss_bc, bss_bc)
                    nc.vector.tensor_scalar(scpk[:], scpk[:], ss_bc[:, 0:1],
                                            None, op0=AL.mult)
                    nc.vector.tensor_scalar(scpk[:], scpk[:], smn_bc[:, 0:1],
                                            None, op0=AL.add)
                    return scpk

                qscale1 = rangebn_scale(cstat, invs1_bc, bias1_bc, s1_bc,
                                        mn1_bc, 'bn1')
                A1 = perm.tile([P, G], f32, name='A1')
                nc.vector.tensor_mul(A1[:], qscale1[:], qbn1w_t[:])
                cA1 = perm.tile([P, G], f32, name='cA1')
                nc.vector.tensor_scalar(cA1[:], A1[:], s1_bc[:, 0:1], None,
                                        op0=AL.mult)

                # mean1 = (sum over cores+batches of h1 sums) / N_TOT
                mean1 = perm.tile([P, G], f32, name='mean1')
                for g in range(G):
                    fsum = Ag[g].rearrange('p s core q b -> p s (core q b)')
                    nc.vector.tensor_reduce(mean1[:, g:g + 1],
                                            fsum[:, 2, :], axis=AX.X, op=AL.add)
                nc.vector.tensor_scalar(mean1[:], mean1[:], 1.0 / N_TOT, None,
                                        op0=AL.mult)
                cB1 = perm.tile([P, G], f32, name='cB1')
                nc.vector.tensor_scalar(cB1[:], mean1[:], -1.0,
                                        mn1_bc[:, 0:1], op0=AL.mult, op1=AL.add)
                # cB1 currently = (mn1 - mean1); multiply by A1, add bn1b
                nc.vector.tensor_mul(cB1[:], cB1[:], A1[:])
                nc.vector.tensor_add(cB1[:], cB1[:], bn1b_t[:])

                # analytic qm(h2) bounds from Ag (monotone: cA1 >= 0)
                for g in range(G):
                    flat = Ag[g].rearrange('p s core q b -> p (s core q b)')
                    ext = flat[:, 0:2 * B_FULL]
                    qchain_small(ext, invs1_bc, bias1_bc)
                    nc.scalar.activation(ext, ext,
                                         AF.Relu, bias=cB1[:, g:g + 1],
                                         scale=cA1[:, g:g + 1])
                q2 = sample_params(Ag, 'h2')
                invs2_bc = bcast(q2['inv_s'], 'invs2_bc')
                bias2_bc = bcast(q2['bias'], 'bias2_bc')
                s2_bc = bcast(q2['s'], 's2_bc')
                mn2_bc = bcast(q2['mn'], 'mn2_bc')

                # E-stage fused affine: k2 = round(clip(aE*k1 + bE, 0, 255))
                aE = perm.tile([P, G], f32, name='aE')
                bE = perm.tile([P, G], f32, name='bE')
                nc.vector.tensor_scalar(aE[:], cA1[:], invs2_bc[:, 0:1], None,
                                        op0=AL.mult)
                nc.vector.tensor_scalar(bE[:], cB1[:], mn2_bc[:, 0:1],
                                        invs2_bc[:, 0:1],
                                        op0=AL.subtract, op1=AL.mult)

                # scaled pointwise weights (bf16: k2 integers exact, weight
                # rounding in the 256-way contraction is harmless) + const3
                pwTs = perm.tile([P, G, 256], bf16, name='pwTs')
                nc.vector.tensor_scalar(pwTs[:], pwT[:], s2_bc[:, 0:1], None,
                                        op0=AL.mult)
                const3 = perm.tile([P, G], f32, name='const3')
                nc.vector.tensor_scalar(const3[:], pwsum_t[:], mn2_bc[:, 0:1],
                                        None, op0=AL.mult)

                # =========================================================
                # Stages D+E+F per batch: h1 -> k1(u8) -> k2(u8) -> bf16
                # -> h3 (SBUF); Sum(h3) accumulated during eviction
                # =========================================================
                _tp2.__exit__(None, None, None)
                h3 = {}
                with tc.tile_pool(name='pw', bufs=8, space='PSUM') as pwp:
                    for b in range(BL):
                        k2b = {}
                        for g in range(G):
                            # D: k1 = round(clip((h1-mn1)/s1)) via the
                            # saturating RNE u8 convert -- one ACT op
                            kt = kpool.tile([P, IMG], u8, name=f'k1_{g}_{b}',
                                            tag='k8')
                            nc.scalar.activation(kt[:], h1[(g, b)][:], AF.Relu,
                                                 bias=bias1_bc[:, 0:1],
                                                 scale=invs1_bc[:, 0:1])
                            # E: k2 = round(clip(aE*k1 + bE)), in place u8->u8
                            nc.scalar.activation(kt[:], kt[:], AF.Relu,
                                                 bias=bE[:, g:g + 1],
                                                 scale=aE[:, g:g + 1])
                            # convert to bf16 for the pointwise matmuls
                            k2t = k2p.tile([P, IMG], bf16, name=f'k2_{g}_{b}',
                                           tag='k2')
                            nc.vector.tensor_scalar(k2t[:], kt[:], 1.0, None,
                                                    op0=AL.mult)
                            k2b[g] = k2t
                        # F: pointwise conv for this batch (bf16)
                        for cg in range(G):
                            h3t = img.tile([P, IMG], f32, name=f'h3_{cg}_{b}',
                                           tag='img')
                            h3[(cg, b)] = h3t
                            for blk in (PW_CHUNKS[0:4], PW_CHUNKS[4:7]):
                                pst = {}
                                for (c0, nn) in blk:
                                    pst[c0] = pwp.tile([P, PWC], f32,
                                                       name=f'pw{cg}{b}{c0}',
                                                       tag='pw')
                                for kg in range(G):
                                    lhs = pwTs[:, kg, cg * P:(cg + 1) * P]
                                    for (c0, nn) in blk:
                                        nc.tensor.matmul(
                                            pst[c0][:, 0:nn], lhs,
                                            k2b[kg][:, c0:c0 + nn],
                                            start=(kg == 0), stop=(kg == 1))
                                for ji, (c0, nn) in enumerate(blk):
                                    j = (0 if c0 < 4 * PWC else 4) + ji
                                    nc.scalar.activation(
                                        h3t[:, c0:c0 + nn], pst[c0][:, 0:nn],
                                        AF.Identity, bias=const3[:, cg:cg + 1],
                                        scale=1.0,
                                        accum_out=hsum8[cg][:, b, j:j + 1])
                            minmax_tree(f'h3s{cg}_{b}', h3t[:],
                                        h3stat[cg][:, b:b + 1],
                                        h3stat[cg][:, BL + b:BL + b + 1])
                            nc.vector.tensor_reduce(
                                h3stat[cg][:, 2 * BL + b:2 * BL + b + 1],
                                hsum8[cg][:, b, 0:7], axis=AX.X, op=AL.add)

                # =========================================================
                # AG5 + RangeBN2 stats
                # =========================================================
                _tp3 = tc.tile_pool(name='tp3', bufs=2, space='PSUM')
                tpp = _tp3.__enter__()
                ag5_in = dpool.tile([G * P * 3 * BL], f32, name='ag5_in')
                ag5_out = dpool.tile([NCORES * G * P * 3 * BL], f32,
                                     name='ag5_out')
                v5i = ag5_in.rearrange('(g c f) -> g c f', g=G, c=P)
                for g in range(G):
                    nc.sync.dma_start(v5i[g], h3stat[g][:])
                nc.gpsimd.collective_compute(
                    'AllGather', AL.bypass, replica_groups=rg,
                    ins=[ag5_in[:].opt()], outs=[ag5_out[:].opt()])
                v5o = ag5_out.rearrange(
                    '(core g c s q b) -> g c s core q b',
                    core=NCORES, g=G, c=P, s=3, q=2)
                for g in range(G):
                    for s in range(3):
                        nc.sync.dma_start(Ag[g][:, s], v5o[g][:, s])
                for g in range(G):
                    nc.vector.tensor_reduce(
                        cstat[g][:, 0, :], Ag[g][:, 0], axis=AX.X, op=AL.min)
                    nc.vector.tensor_reduce(
                        cstat[g][:, 1, :], Ag[g][:, 1], axis=AX.X, op=AL.max)
                q3 = sample_params(Ag, 'h3')
                invs3_bc = bcast(q3['inv_s'], 'invs3_bc')
                bias3_bc = bcast(q3['bias'], 'bias3_bc')
                s3_bc = bcast(q3['s'], 's3_bc')
                mn3_bc = bcast(q3['mn'], 'mn3_bc')
                qscale3 = rangebn_scale(cstat, invs3_bc, bias3_bc, s3_bc,
                                        mn3_bc, 'bn2')
                A3 = perm.tile([P, G], f32, name='A3')
                nc.vector.tensor_mul(A3[:], qscale3[:], qbn2w_t[:])
                cA3 = perm.tile([P, G], f32, name='cA3')
                nc.vector.tensor_scalar(cA3[:], A3[:], s3_bc[:, 0:1], None,
                                        op0=AL.mult)
                mean3 = perm.tile([P, G], f32, name='mean3')
                for g in range(G):
                    fsum = Ag[g].rearrange('p s core q b -> p s (core q b)')
                    nc.vector.tensor_reduce(mean3[:, g:g + 1],
                                            fsum[:, 2, :], axis=AX.X, op=AL.add)
                nc.vector.tensor_scalar(mean3[:], mean3[:], 1.0 / N_TOT, None,
                                        op0=AL.mult)
                cB3 = perm.tile([P, G], f32, name='cB3')
                nc.vector.tensor_scalar(cB3[:], mean3[:], -1.0,
                                        mn3_bc[:, 0:1], op0=AL.mult, op1=AL.add)
                nc.vector.tensor_mul(cB3[:], cB3[:], A3[:])
                nc.vector.tensor_add(cB3[:], cB3[:], bn2b_t[:])

                _tp3.__exit__(None, None, None)
                # =========================================================
                # Stages G+H per tile: h3 -> k3 (in place) -> out -> DMA
                # =========================================================
                for b in range(BL):
                    for g in range(G):
                        # G: k3 = round(clip((h3-mn3)/s3)) via u8 convert
                        kt3 = kpool.tile([P, IMG], u8, name=f'k3_{g}_{b}',
                                         tag='k8')
                        nc.scalar.activation(kt3[:], h3[(g, b)][:], AF.Relu,
                                             bias=bias3_bc[:, 0:1],
                                             scale=invs3_bc[:, 0:1])
                        # H on DVE: out = relu(cA3*k3 + cB3)
                        ot = img.tile([P, IMG], f32, name=f'out_{g}_{b}',
                                      tag='img')
                        nc.vector.tensor_scalar(ot[:], kt3[:],
                                                cA3[:, g:g + 1],
                                                cB3[:, g:g + 1],
                                                op0=AL.mult, op1=AL.add)
                        nc.vector.tensor_scalar(ot[:], ot[:], 0.0, None,
                                                op0=AL.max)
                        nc.sync.dma_start(
                            out_d[b, g * P:(g + 1) * P].rearrange(
                                'c h w -> c (h w)'), ot[:])

    nc.compile()
    return nc


def _host_consts(dw_w, dw_b, bn1_w, bn1_b, pw_w, bn2_w, bn2_b):
    qdw = _host_quant(dw_w).reshape(256, 9)
    qdb = _host_quant(dw_b)
    qpw = _host_quant(pw_w).reshape(256, 256)
    qbn1w = _host_quant(bn1_w)
    qbn2w = _host_quant(bn2_w)
    wsum = qdw.sum(axis=1, dtype=np.float32)
    pwsum = qpw.sum(axis=1, dtype=np.float32)
    # lhsT layout: pwT[kg, cin, (coutg*128 + cout)] = qpw[cout_full, kg*128+cin]
    pwT = np.ascontiguousarray(
        qpw.T.reshape(G, P, 256)).astype(np.float32)
    consts = {
        'ident': np.eye(P, dtype=np.float32),
        'qdw': np.ascontiguousarray(qdw.reshape(G, P, 9)),
        'wsum': wsum.reshape(G, P).copy(),
        'qdb': qdb.reshape(G, P).copy(),
        'qbn1w': qbn1w.reshape(G, P).copy(),
        'bn1b': np.asarray(bn1_b, np.float32).reshape(G, P).copy(),
        'qbn2w': qbn2w.reshape(G, P).copy(),
        'bn2b': np.asarray(bn2_b, np.float32).reshape(G, P).copy(),
        'pwsum': pwsum.reshape(G, P).copy(),
        'pwT': pwT,
    }
    return consts


def make_in_maps(x, dw_w, dw_b, bn1_w, bn1_b, pw_w, bn2_w, bn2_b):
    x = np.asarray(x, np.float32)
    consts = _host_consts(dw_w, dw_b, bn1_w, bn1_b, pw_w, bn2_w, bn2_b)
    in_maps = []
    for c in range(NCORES):
        m = dict(consts)
        m['x'] = np.ascontiguousarray(x[c * BL:(c + 1) * BL])
        in_maps.append(m)
    return in_maps


def get_program(limit=7):
    if limit not in _PROGRAM_CACHE:
        _PROGRAM_CACHE[limit] = build_program(limit)
    return _PROGRAM_CACHE[limit]


def kernel(**inputs):
    from concourse.bass_utils import run_bass_kernel_spmd
    nc = get_program()
    in_maps = make_in_maps(**inputs)
    res = run_bass_kernel_spmd(nc, in_maps, core_ids=list(range(NCORES)))
    out = np.concatenate([res.results[i]['out'] for i in range(NCORES)],
                         axis=0)
    return out.astype(np.float32)
